# revision 9
# baseline (speedup 1.0000x reference)
"""GAT (2-layer, 8-head, mean over heads) Trainium2 Bass kernel, 8-core SPMD.

v3: dst-range sharding with dma_gather-based edge gathers (HW-verified
semantics; the v2 batched indirect-DMA turned out to stream contiguous rows
on HW). Design:

  * Per-node record tables in fp8 (h) with f32 alpha columns bit-cast into
    fp8 slots. Row strides are 256B-multiples (dma_gather constraint):
      hs1:   [h fp8 x512 | a_src f32 x8 @byte 512 | pad]    row 768 B
      hs2:   [h fp8 x256 | a_src f32 x8 @byte 256 | pad]    row 512 B
      adown: [a_dst f32 x8 | pad]                           row 256 B
  * dma_gather uses int16 indices (<32768), so the per-edge src gather is
    split into a lo-rows and hi-rows gather against two row-range views of
    the table; per-block chunk counts for each half are compile-time
    constants (max over cores).
  * Layer-1 tables are built in a PER-CORE PERMUTED row order (own dst range
    first, then the rest in natural order; the host permutes xT's columns),
    which makes "own rows" positional -> the dst-alpha table adown1 and its
    local (int16) dst indices are SPMD-clean.
  * Layer-2 records for the own dst range are computed from the transposed
    layer-1 activations and exchanged with one AllGather (natural row order);
    adown2 is written locally before the AllGather.
  * Per-block (not per-chunk) DVE/ACT processing; selection matrix S built
    with an int8 compare; scatter + denominators via per-chunk matmuls
    accumulating in PSUM.
"""

import os
import ml_dtypes
import numpy as np
from contextlib import ExitStack

N = 50000
E = 800000
H = 8
IN = 256
O1 = 64
O2 = 32
NCORE = 8
P = 128
NEG = 0.2
LOHALF = 32768

ROW1 = 640           # hs1 row, bf16 elements (1280 B)
ROW2 = 384           # hs2 row (768 B)
ROWA = 128           # adown row (256 B)
F1 = H * O1          # 512
F2 = H * O2          # 256

_cached = {}


def _wrap_idx(flat, nidx):
    """dma_gather index layout: [128, nidx//16] int16, idx j at
    [j%16, j//16], replicated to all 8 groups of 16 partitions."""
    a = np.zeros((16, nidx // 16), np.int16)
    a[:, :] = np.asarray(flat, np.int16).reshape(nidx // 16, 16).T
    return np.tile(a, (8, 1))


def _build_meta(edge_index, n, ncore, lohalf):
    ndst = n // ncore
    nblk = (ndst + P - 1) // P
    split = (nblk // 2) * P          # local-row split for the two L2 tables
    src = np.concatenate([edge_index[0], np.arange(n, dtype=np.int64)])
    dst = np.concatenate([edge_index[1], np.arange(n, dtype=np.int64)])

    # per-core sorted edges + per-layer lo/hi split counts
    edges = []          # [core][block] -> (srow1, srow2, dloc) arrays
    nlo = np.zeros((2, ncore, nblk), np.int64)
    nhi = np.zeros((2, ncore, nblk), np.int64)
    for k in range(ncore):
        lo = k * ndst
        m = (dst >= lo) & (dst < lo + ndst)
        s_k = src[m]
        d_k = dst[m] - lo
        o = np.argsort(d_k, kind="stable")
        s_k = s_k[o]
        d_k = d_k[o]
        pos = np.zeros(n, np.int64)
        pos[lo:lo + ndst] = np.arange(ndst)
        others = np.concatenate([np.arange(0, lo), np.arange(lo + ndst, n)])
        pos[others] = ndst + np.arange(n - ndst)
        srow1 = pos[s_k]
        # L2 row ids in the A/B split tables: node g -> core k_g, local i;
        # A rows: k_g*split + i (i < split); B rows: k_g*(ndst-split) + i-split
        kg = s_k // ndst
        ig = s_k % ndst
        inA = ig < split
        srow2 = np.where(inA, kg * split + ig,
                         kg * (ndst - split) + (ig - split))
        blk = d_k // P
        per = []
        for b in range(nblk):
            mb = blk == b
            s1, s2, dl = srow1[mb], srow2[mb], d_k[mb]
            lo1 = s1 < lohalf
            lo2 = inA[mb]
            nlo[0, k, b] = lo1.sum()
            nhi[0, k, b] = (~lo1).sum()
            nlo[1, k, b] = lo2.sum()
            nhi[1, k, b] = (~lo2).sum()
            per.append((s1, s2, dl, lo2))
        edges.append(per)

    cdiv = lambda a, b: -(-a // b)
    CLO = [[int(cdiv(nlo[L, :, b].max(), P)) for b in range(nblk)]
           for L in (0, 1)]
    CHI = [[int(cdiv(nhi[L, :, b].max(), P)) for b in range(nblk)]
           for L in (0, 1)]
    CBb = [[CLO[L][b] + CHI[L][b] for b in range(nblk)] for L in (0, 1)]

    # idx tile column layouts (shared across cores)
    scol = [np.cumsum([0] + [(CLO[L][b] + CHI[L][b]) * 8 for b in range(nblk)])
            for L in (0, 1)]
    acol = [np.cumsum([0] + [CBb[L][b] * 8 for b in range(nblk)])
            for L in (0, 1)]

    CBMAX = max(max(CBb[0]), max(CBb[1]))
    metas = []
    for k in range(ncore):
        per_layer = {}
        for L in (0, 1):
            sidx = np.zeros((P, scol[L][-1]), np.int16)
            lrow = np.full((nblk, CBMAX * P), 200.0, np.float32)
            ld = np.full((P, sum(CBb[L])), -1, np.int8)
            ldoff = np.cumsum([0] + CBb[L])
            for b in range(nblk):
                s1, s2, dl, inA_b = edges[k][b]
                srow = s1 if L == 0 else s2
                isl = (srow < lohalf) if L == 0 else inA_b
                cl, ch = CLO[L][b], CHI[L][b]
                sl = np.zeros(cl * P, np.int64)
                sh = np.zeros(ch * P, np.int64)
                sl[:isl.sum()] = srow[isl]
                sh[:(~isl).sum()] = (srow[~isl] - lohalf) if L == 0 \
                    else srow[~isl]
                dfull = np.full((cl + ch) * P, -1, np.int64)
                dfull[:isl.sum()] = dl[isl]
                dfull[cl * P:cl * P + (~isl).sum()] = dl[~isl]
                c0 = scol[L][b]
                if cl:
                    sidx[:, c0:c0 + cl * 8] = _wrap_idx(sl, cl * P)
                if ch:
                    sidx[:, c0 + cl * 8:c0 + (cl + ch) * 8] = _wrap_idx(
                        sh, ch * P)
                lb = dfull - b * P
                lrow[b, 0:(cl + ch) * P] = np.where(lb >= 0, lb, 200)
                lb2 = lb.copy()
                lb2[dfull < 0] = -1
                ld[:, ldoff[b]:ldoff[b + 1]] = lb2.reshape(cl + ch, P).T
            per_layer[L] = (sidx, lrow, ld)
        metas.append(per_layer)
    key = (tuple(CLO[0]), tuple(CHI[0]), tuple(CLO[1]), tuple(CHI[1]))
    return key, CLO, CHI, metas


def _build_program(CLO, CHI, n, ncore, lohalf):
    import concourse.bacc as bacc
    import concourse.tile as tile
    from concourse import bass, mybir

    f32 = mybir.dt.float32
    bf16 = mybir.dt.bfloat16
    i16 = mybir.dt.int16
    i8 = mybir.dt.int8
    i32 = mybir.dt.int32
    AL = mybir.AluOpType
    AF = mybir.ActivationFunctionType

    ndst = n // ncore
    nblk = (ndst + P - 1) // P
    CBb = [[CLO[L][b] + CHI[L][b] for b in range(nblk)] for L in (0, 1)]
    scol = [np.cumsum([0] + [(CLO[L][b] + CHI[L][b]) * 8
                             for b in range(nblk)]) for L in (0, 1)]
    acol = [np.cumsum([0] + [CBb[L][b] * 8 for b in range(nblk)])
            for L in (0, 1)]
    ldoff = [np.cumsum([0] + CBb[L]) for L in (0, 1)]
    CBMAX = max(max(CBb[0]), max(CBb[1]))

    nc = bacc.Bacc("TRN2", target_bir_lowering=False, debug=False,
                   enable_asserts=True, num_devices=ncore,
                   num_swdge_queues=4)
    xT_d = nc.dram_tensor("xT", [IN, n], bf16, kind="ExternalInput")
    w1h_d = nc.dram_tensor("w1h", [IN, F1], bf16, kind="ExternalInput")
    w1a_d = nc.dram_tensor("w1a", [IN, 16], bf16, kind="ExternalInput")
    w2_d = nc.dram_tensor("w2cat", [O1, 16 + F2], f32, kind="ExternalInput")
    b1_d = nc.dram_tensor("b1rep", [P, O1], f32, kind="ExternalInput")
    b2_d = nc.dram_tensor("b2rep", [P, O2], f32, kind="ExternalInput")
    si1_d = nc.dram_tensor("sidx1", [P, int(scol[0][-1])], i16,
                           kind="ExternalInput")
    lr1_d = nc.dram_tensor("lr1", [nblk, CBMAX * P], bf16,
                           kind="ExternalInput")
    ld1_d = nc.dram_tensor("ld1", [P, int(ldoff[0][-1])], i8,
                           kind="ExternalInput")
    si2_d = nc.dram_tensor("sidx2", [P, int(scol[1][-1])], i16,
                           kind="ExternalInput")
    lr2_d = nc.dram_tensor("lr2", [nblk, CBMAX * P], bf16,
                           kind="ExternalInput")
    ld2_d = nc.dram_tensor("ld2", [P, int(ldoff[1][-1])], i8,
                           kind="ExternalInput")
    outf_d = nc.dram_tensor("outf", [ndst, O2], f32, kind="ExternalOutput")
    hs1 = nc.dram_tensor("hs1", [n, ROW1], bf16)
    ad1t = nc.dram_tensor("ad1t", [ndst, ROWA], bf16)
    ad2t = nc.dram_tensor("ad2t", [ndst, ROWA], bf16)
    split = (nblk // 2) * P
    shr = "Shared" if ncore > 4 else "Local"
    hs2ownA = nc.dram_tensor("hs2ownA", [split, ROW2], bf16)
    hs2ownB = nc.dram_tensor("hs2ownB", [ndst - split, ROW2], bf16)
    hs2A = nc.dram_tensor("hs2A", [ncore * split, ROW2], bf16, addr_space=shr)
    hs2B = nc.dram_tensor("hs2B", [ncore * (ndst - split), ROW2], bf16,
                          addr_space=shr)

    with tile.TileContext(nc) as tc, ExitStack() as ctx:
        cpool = ctx.enter_context(tc.tile_pool(name="const", bufs=1))

        iota_i = cpool.tile([P, P], i32, tag="io_i")
        nc.gpsimd.iota(iota_i[:], pattern=[[1, P]], base=0, channel_multiplier=0)
        iota8 = cpool.tile([P, P], i8, tag="io_8")
        nc.vector.tensor_copy(iota8[:], iota_i[:])
        iotaF = cpool.tile([P, CBMAX * P], i8, tag="iotaF")
        for c in range(CBMAX):
            nc.vector.tensor_copy(iotaF[:, c * P:(c + 1) * P], iota8[:])
        iotac_i = cpool.tile([P, 1], i32, tag="ioc_i")
        nc.gpsimd.iota(iotac_i[:], pattern=[[1, 1]], base=0, channel_multiplier=1)
        iotacf = cpool.tile([P, 1], f32, tag="ioc_f")
        nc.vector.tensor_copy(iotacf[:], iotac_i[:])
        iotaff = cpool.tile([P, P], f32, tag="io_f")
        nc.vector.tensor_copy(iotaff[:], iota_i[:])
        ident = cpool.tile([P, P], f32, tag="ident")
        nc.vector.tensor_scalar(out=ident[:], in0=iotaff[:], scalar1=iotacf[:, 0:1],
                                scalar2=None, op0=AL.is_equal)
        b1s = cpool.tile([P, O1], f32, tag="b1")
        nc.sync.dma_start(out=b1s[:], in_=b1_d.ap()[:, :])
        b2s = cpool.tile([P, O2], f32, tag="b2")
        nc.sync.dma_start(out=b2s[:], in_=b2_d.ap()[:, :])
        xt2sb = cpool.tile([O1, nblk * P], f32, tag="xt2")
        ones_row = cpool.tile([1, P], bf16, tag="ones")
        nc.vector.memset(ones_row[:], 1.0)
        w2s = cpool.tile([O1, 16 + F2], f32, tag="w2")
        nc.sync.dma_start(out=w2s[:], in_=w2_d.ap()[:, :])

        # ---------------- phase A1: layer-1 records (permuted order) --------
        with tc.tile_pool(name="pa_x", bufs=2) as xp, \
             tc.tile_pool(name="pa_w", bufs=1) as wp, \
             tc.tile_pool(name="pa_rec", bufs=3) as rp, \
             tc.tile_pool(name="pa_adr", bufs=3) as arp, \
             tc.tile_pool(name="pa_pm", bufs=3, space="PSUM") as pmp, \
             tc.tile_pool(name="pa_pa", bufs=3, space="PSUM") as pap:
            w1ha = wp.tile([P, F1], bf16, tag="w1ha")
            nc.sync.dma_start(out=w1ha[:], in_=w1h_d.ap()[0:P, :])
            w1hb = wp.tile([P, F1], bf16, tag="w1hb")
            nc.sync.dma_start(out=w1hb[:], in_=w1h_d.ap()[P:IN, :])
            w1aa = wp.tile([P, 16], bf16, tag="w1aa")
            nc.sync.dma_start(out=w1aa[:], in_=w1a_d.ap()[0:P, :])
            w1ab = wp.tile([P, 16], bf16, tag="w1ab")
            nc.sync.dma_start(out=w1ab[:], in_=w1a_d.ap()[P:IN, :])
            CHK = 2048
            for g0 in range(0, n, CHK):
                gw = min(CHK, n - g0)
                xa = xp.tile([P, CHK], bf16, tag="xa")
                nc.sync.dma_start(out=xa[:, :gw], in_=xT_d.ap()[0:P, g0:g0 + gw])
                xb = xp.tile([P, CHK], bf16, tag="xb")
                nc.sync.dma_start(out=xb[:, :gw], in_=xT_d.ap()[P:IN, g0:g0 + gw])
                for off in range(0, gw, P):
                    m = min(P, gw - off)
                    row0 = g0 + off
                    psh = pmp.tile([P, F1], f32, tag="psh")
                    nc.tensor.matmul(psh[:m, :], lhsT=xa[:, off:off + m],
                                     rhs=w1ha[:, :], start=True, stop=False)
                    nc.tensor.matmul(psh[:m, :], lhsT=xb[:, off:off + m],
                                     rhs=w1hb[:, :], start=False, stop=True)
                    psa = pap.tile([P, 16], f32, tag="psa")
                    nc.tensor.matmul(psa[:m, :], lhsT=xa[:, off:off + m],
                                     rhs=w1aa[:, :], start=True, stop=False)
                    nc.tensor.matmul(psa[:m, :], lhsT=xb[:, off:off + m],
                                     rhs=w1ab[:, :], start=False, stop=True)
                    rec = rp.tile([P, F1 + 16], bf16, tag="rec")
                    nc.scalar.activation(out=rec[:m, 0:F1], in_=psh[:m, :],
                                         func=AF.Copy)
                    recf = rec[:].bitcast(f32)
                    nc.vector.tensor_copy(recf[:m, 256:256 + H], psa[:m, 0:H])
                    nc.sync.dma_start(out=hs1.ap()[row0:row0 + m, 0:F1 + 16],
                                      in_=rec[:m, :])
                    if row0 < ndst:
                        mm = min(m, ndst - row0)
                        adr = arp.tile([P, 16], bf16, tag="adr")
                        adrf = adr[:].bitcast(f32)
                        nc.vector.tensor_copy(adrf[:mm, 0:H], psa[:mm, H:2 * H])
                        nc.sync.dma_start(out=ad1t.ap()[row0:row0 + mm, 0:16],
                                          in_=adr[:mm, :])

        # ---------------- edge phase ----------------------------------------
        qctr = [0]

        def edge_phase(layer):
            L = layer - 1
            if layer == 1:
                ROW, Fh, F, adt = ROW1, O1, F1, ad1t
                si_d_, lr_d_, ld_d_ = si1_d, lr1_d, ld1_d
                ASF = 256            # f32 index of a_src in row
                viewlo = hs1.ap()[0:lohalf, :]
                viewhi = hs1.ap()[lohalf:n, :]
            else:
                ROW, Fh, F, adt = ROW2, O2, F2, ad2t
                si_d_, lr_d_, ld_d_ = si2_d, lr2_d, ld2_d
                ASF = 128
                viewlo = hs2A.ap()[:, :]
                viewhi = hs2B.ap()[:, :]
            RF = ROW // 4            # f32 elements per row
            with tc.tile_pool(name=f"ep{layer}_idx", bufs=1) as idxp, \
                 tc.tile_pool(name=f"ep{layer}_rec", bufs=2) as recp, \
                 tc.tile_pool(name=f"ep{layer}_adv", bufs=2) as advp, \
                 tc.tile_pool(name=f"ep{layer}_S", bufs=2) as sp, \
                 tc.tile_pool(name=f"ep{layer}_sm", bufs=2) as smp, \
                 tc.tile_pool(name=f"ep{layer}_msg", bufs=2) as msgp, \
                 tc.tile_pool(name=f"ep{layer}_epi", bufs=2) as epi, \
                 tc.tile_pool(name=f"ep{layer}_lr", bufs=2) as lrp, \
                 tc.tile_pool(name=f"ep{layer}_St", bufs=2) as stp, \
                 tc.tile_pool(name=f"ep{layer}_out", bufs=2, space="PSUM") as outp_, \
                 tc.tile_pool(name=f"ep{layer}_den", bufs=2, space="PSUM") as denp, \
                 tc.tile_pool(name=f"ep{layer}_tr", bufs=1, space="PSUM") as trp, \
                 tc.tile_pool(name=f"ep{layer}_ade", bufs=2, space="PSUM") as adep, \
                 tc.tile_pool(name=f"ep{layer}_ps2", bufs=1, space="PSUM") as pp2:
                si_sb = idxp.tile([P, int(scol[L][-1])], i16, tag="si")
                nc.sync.dma_start(out=si_sb[:], in_=si_d_.ap()[:, :])
                ld_sb = idxp.tile([P, int(ldoff[L][-1])], i8, tag="ld")
                nc.sync.dma_start(out=ld_sb[:], in_=ld_d_.ap()[:, :])
                for b in range(nblk):
                    bbase = b * P
                    bm = min(P, ndst - bbase)
                    CL, CH = CLO[L][b], CHI[L][b]
                    CB = CL + CH
                    lo0 = int(ldoff[L][b])
                    rec = recp.tile([P, CB * ROW], bf16, tag="rec")
                    s0 = int(scol[L][b])
                    if CL:
                        nc.gpsimd.dma_gather(
                            rec[:, 0:CL * ROW].rearrange("p (c r) -> p c r",
                                                         r=ROW),
                            viewlo,
                            si_sb[:, s0:s0 + CL * 8],
                            CL * P, CL * P, ROW, single_packet=False,
                            queue_num=qctr[0] % 4)
                        qctr[0] += 1
                    if CH:
                        nc.gpsimd.dma_gather(
                            rec[:, CL * ROW:CB * ROW].rearrange(
                                "p (c r) -> p c r", r=ROW),
                            viewhi,
                            si_sb[:, s0 + CL * 8:s0 + CB * 8],
                            CH * P, CH * P, ROW, single_packet=False,
                            queue_num=qctr[0] % 4)
                        qctr[0] += 1
                    # dst alphas: contiguous local rows -> plain DMA + bcast
                    adv = advp.tile([P, 16], bf16, tag="adv")
                    nc.sync.dma_start(out=adv[:bm, :],
                                      in_=adt.ap()[bbase:bbase + bm, 0:16])
                    adb = advp.tile([P, H], bf16, tag="adb")
                    nc.scalar.activation(out=adb[:], in_=adv[:].bitcast(f32),
                                         func=AF.Copy)
                    lrow = lrp.tile([P, CBMAX * P], bf16, tag="lrow")
                    nc.sync.dma_start(
                        out=lrow[:, 0:CB * P],
                        in_=lr_d_.ap()[b:b + 1, 0:CB * P]
                            .partition_broadcast(P))
                    St = stp.tile([P, CB * P], bf16, tag="St")
                    nc.gpsimd.tensor_scalar(out=St[:, 0:CB * P],
                                            in0=lrow[:, 0:CB * P],
                                            scalar1=iotacf[:, 0:1],
                                            scalar2=None, op0=AL.is_equal)
                    ade = adep.tile([P, CB * H], f32, tag="ade")
                    for c in range(CB):
                        nc.tensor.matmul(ade[:, c * H:(c + 1) * H],
                                         lhsT=St[:, c * P:(c + 1) * P],
                                         rhs=adb[:], start=True, stop=True)
                    S = sp.tile([P, CB * P], bf16, tag="S")
                    nc.vector.tensor_tensor(
                        out=S[:].rearrange("p (c j) -> p c j", c=CB),
                        in0=iotaF[:, 0:CB * P].rearrange("p (c j) -> p c j",
                                                         c=CB),
                        in1=ld_sb[:, lo0:lo0 + CB].to_broadcast([P, CB, P]),
                        op=AL.is_equal)
                    recf = rec[:].bitcast(f32)
                    et = smp.tile([P, CB * H], f32, tag="et")
                    nc.vector.tensor_tensor(
                        out=et[:].rearrange("p (c h) -> p c h", c=CB),
                        in0=recf.rearrange("p (c r) -> p c r", c=CB)
                            [:, :, ASF:ASF + H],
                        in1=ade[:].rearrange("p (c h) -> p c h", c=CB),
                        op=AL.add)
                    lt = smp.tile([P, CB * H], f32, tag="lt")
                    nc.vector.tensor_scalar(out=lt[:], in0=et[:], scalar1=NEG,
                                            scalar2=None, op0=AL.mult)
                    nc.vector.tensor_tensor(out=lt[:], in0=lt[:], in1=et[:],
                                            op=AL.max)
                    ex = smp.tile([P, CB * H], f32, tag="ex")
                    nc.scalar.activation(out=ex[:], in_=lt[:], func=AF.Exp)
                    exb = smp.tile([P, CB * H], bf16, tag="exb")
                    nc.scalar.activation(out=exb[:], in_=ex[:], func=AF.Copy)
                    msgb = msgp.tile([P, CB * F], bf16, tag="msgb")
                    nc.vector.tensor_tensor(
                        out=msgb[:].rearrange("p (c h f) -> p c h f",
                                              c=CB, h=H),
                        in0=rec[:].rearrange("p (c r) -> p c r", c=CB)
                            [:, :, 0:F].rearrange("p c (h f) -> p c h f", h=H),
                        in1=ex[:].rearrange("p (c h) -> p c h", c=CB)
                            .to_broadcast([P, CB, H, Fh]),
                        op=AL.mult)
                    outp = outp_.tile([P, F], f32, tag="out")
                    den = denp.tile([P, H], f32, tag="den")
                    for c in range(CB):
                        nc.tensor.matmul(outp[:], lhsT=S[:, c * P:(c + 1) * P],
                                         rhs=msgb[:, c * F:(c + 1) * F],
                                         start=(c == 0), stop=(c == CB - 1))
                        nc.tensor.matmul(den[:], lhsT=S[:, c * P:(c + 1) * P],
                                         rhs=exb[:, c * H:(c + 1) * H],
                                         start=(c == 0), stop=(c == CB - 1))
                    r = epi.tile([P, H], f32, tag="r")
                    nc.vector.tensor_scalar(out=r[:], in0=den[:], scalar1=1e-16,
                                            scalar2=None, op0=AL.add)
                    nc.vector.reciprocal(r[:], r[:])
                    nc.vector.tensor_scalar(out=r[:], in0=r[:], scalar1=1.0 / H,
                                            scalar2=None, op0=AL.mult)
                    tmp0 = epi.tile([P, F], f32, tag="tmp0")
                    nc.scalar.activation(out=tmp0[:], in_=outp[:], func=AF.Copy)
                    tmp = epi.tile([P, F], f32, tag="tmp")
                    nc.vector.tensor_tensor(
                        out=tmp[:].rearrange("p (h f) -> p h f", h=H),
                        in0=tmp0[:].rearrange("p (h f) -> p h f", h=H),
                        in1=r[:].to_broadcast([P, H, Fh]),
                        op=AL.mult)
                    acc = epi.tile([P, Fh], f32, tag="acc")
                    nc.vector.tensor_reduce(
                        out=acc[:], in_=tmp[:].rearrange("p (h f) -> p f h",
                                                         h=H),
                        axis=mybir.AxisListType.X, op=AL.add)
                    bs = b1s if layer == 1 else b2s
                    nc.vector.tensor_tensor(out=acc[:], in0=acc[:],
                                            in1=bs[:, 0:Fh], op=AL.add)
                    if layer == 1:
                        x2t = epi.tile([P, O1], f32, tag="x2")
                        nc.vector.tensor_scalar(out=x2t[:], in0=acc[:],
                                                scalar1=0.0, scalar2=None,
                                                op0=AL.max)
                        tr = trp.tile([O1, P], f32, tag="tr")
                        nc.tensor.transpose(out=tr[:], in_=x2t[:],
                                            identity=ident[:])
                        nc.vector.tensor_copy(xt2sb[:, bbase:bbase + P], tr[:])
                        ps2 = pp2.tile([P, 16 + F2], f32, tag="ps2")
                        nc.tensor.matmul(ps2[:bm, :],
                                         lhsT=xt2sb[:, bbase:bbase + bm],
                                         rhs=w2s[:, :], start=True, stop=True)
                        rec2 = epi.tile([P, F2 + 16], bf16, tag="rec2")
                        nc.scalar.activation(out=rec2[:bm, 0:F2],
                                             in_=ps2[:bm, 16:16 + F2],
                                             func=AF.Copy)
                        rec2f = rec2[:].bitcast(f32)
                        nc.vector.tensor_copy(rec2f[:bm, 128:128 + H],
                                              ps2[:bm, 0:H])
                        if b < nblk // 2:
                            nc.sync.dma_start(
                                out=hs2ownA.ap()[bbase:bbase + bm, 0:F2 + 16],
                                in_=rec2[:bm, :])
                        else:
                            nc.sync.dma_start(
                                out=hs2ownB.ap()[bbase - split:
                                                 bbase - split + bm,
                                                 0:F2 + 16],
                                in_=rec2[:bm, :])
                        adr2 = epi.tile([P, 16], bf16, tag="adr2")
                        adr2f = adr2[:].bitcast(f32)
                        nc.vector.tensor_copy(adr2f[:bm, 0:H],
                                              ps2[:bm, H:2 * H])
                        nc.sync.dma_start(out=ad2t.ap()[bbase:bbase + bm, 0:16],
                                          in_=adr2[:bm, :])
                        if b == nblk // 2 - 1:
                            if ncore > 1:
                                nc.gpsimd.collective_compute(
                                    "AllGather", mybir.AluOpType.bypass,
                                    replica_groups=[list(range(ncore))],
                                    ins=[hs2ownA.ap().opt()],
                                    outs=[hs2A.ap().opt()])
                            else:
                                nc.sync.dma_start(out=hs2A.ap()[:, :],
                                                  in_=hs2ownA.ap()[:, :])
                    else:
                        f = epi.tile([P, O2], f32, tag="f")
                        nc.vector.tensor_scalar(out=f[:], in0=acc[:],
                                                scalar1=0.0, scalar2=None,
                                                op0=AL.max)
                        nmx = epi.tile([P, 1], f32, tag="nmx")
                        nc.vector.tensor_reduce(out=nmx[:], in_=f[:],
                                                axis=mybir.AxisListType.X,
                                                op=AL.max, negate=True)
                        ef = epi.tile([P, O2], f32, tag="ef")
                        nc.scalar.activation(out=ef[:], in_=f[:], func=AF.Exp,
                                             bias=nmx[:, 0:1])
                        sm = epi.tile([P, 1], f32, tag="sm")
                        nc.vector.tensor_reduce(out=sm[:], in_=ef[:],
                                                axis=mybir.AxisListType.X,
                                                op=AL.add)
                        rs = epi.tile([P, 1], f32, tag="rs")
                        nc.vector.reciprocal(rs[:], sm[:])
                        nc.vector.tensor_scalar(out=ef[:], in0=ef[:],
                                                scalar1=rs[:, 0:1], scalar2=None,
                                                op0=AL.mult)
                        nc.sync.dma_start(out=outf_d.ap()[bbase:bbase + bm, :],
                                          in_=ef[:bm, :])

        edge_phase(1)

        if ncore > 1:
            nc.gpsimd.collective_compute(
                "AllGather", mybir.AluOpType.bypass,
                replica_groups=[list(range(ncore))],
                ins=[hs2ownB.ap().opt()], outs=[hs2B.ap().opt()])
        else:
            nc.sync.dma_start(out=hs2B.ap()[:, :], in_=hs2ownB.ap()[:, :])

        edge_phase(2)

    nc.compile()
    # Align each gather's SWDGE queue with its tile-assigned DMASW sem lane
    # (a sem lane is locked to one queue; lanes are assigned in scheduled
    # order, so the queue must be derived, not chosen up front).
    import re as _re
    for fn in nc.m.functions:
        for bb in fn.blocks:
            for inst in bb.instructions:
                if type(inst).__name__ == "InstDMAGatherAnt":
                    si = inst.sync_info
                    for u in (si.on_update if si is not None else []):
                        mm = _re.match(r"DMASW(\d+)_", u.ant_name or "")
                        if mm:
                            inst.queue_num = int(mm.group(1)) % 4
                            break
    return nc


def _prep_inputs(x, edge_index, W1, a_src1, a_dst1, b1, W2, a_src2, a_dst2, b2,
                 n, ncore, lohalf):
    ndst = n // ncore
    x = np.asarray(x, np.float32)
    W1 = np.asarray(W1, np.float32)
    W2 = np.asarray(W2, np.float32)
    As1 = np.einsum("hf,hfc->ch", np.asarray(a_src1, np.float32),
                    W1.reshape(H, O1, IN)).astype(np.float32)
    Ad1 = np.einsum("hf,hfc->ch", np.asarray(a_dst1, np.float32),
                    W1.reshape(H, O1, IN)).astype(np.float32)
    w1h = np.ascontiguousarray(W1.T)
    w1a = np.ascontiguousarray(np.concatenate([As1, Ad1], axis=1))
    As2 = np.einsum("hf,hfc->ch", np.asarray(a_src2, np.float32),
                    W2.reshape(H, O2, O1)).astype(np.float32)
    Ad2 = np.einsum("hf,hfc->ch", np.asarray(a_dst2, np.float32),
                    W2.reshape(H, O2, O1)).astype(np.float32)
    w2cat = np.ascontiguousarray(
        np.concatenate([As2, Ad2, W2.T], axis=1)).astype(np.float32)
    b1rep = np.ascontiguousarray(
        np.tile(np.asarray(b1, np.float32)[None, :], (P, 1)))
    b2rep = np.ascontiguousarray(
        np.tile(np.asarray(b2, np.float32)[None, :], (P, 1)))

    key, CLO, CHI, metas = _build_meta(np.asarray(edge_index), n, ncore,
                                       lohalf)
    xT = x.T
    in_maps = []
    for k in range(ncore):
        lo = k * ndst
        perm = np.concatenate([np.arange(lo, lo + ndst),
                               np.arange(0, lo), np.arange(lo + ndst, n)])
        sidx1, lrow1, ld1 = metas[k][0]
        sidx2, lrow2, ld2 = metas[k][1]
        in_maps.append({
            "xT": np.ascontiguousarray(xT[:, perm]).astype(ml_dtypes.bfloat16),
            "w1h": w1h.astype(ml_dtypes.bfloat16),
            "w1a": w1a.astype(ml_dtypes.bfloat16),
            "w2cat": w2cat,
            "b1rep": b1rep, "b2rep": b2rep,
            "sidx1": sidx1, "lr1": lrow1.astype(ml_dtypes.bfloat16),
            "ld1": ld1,
            "sidx2": sidx2, "lr2": lrow2.astype(ml_dtypes.bfloat16),
            "ld2": ld2,
        })
    return key, CLO, CHI, in_maps


def kernel(x, edge_index, W1, a_src1, a_dst1, b1, W2, a_src2, a_dst2, b2):
    key, CLO, CHI, in_maps = _prep_inputs(
        x, edge_index, W1, a_src1, a_dst1, b1, W2, a_src2, a_dst2, b2,
        N, NCORE, LOHALF)
    if key not in _cached:
        _cached[key] = _build_program(CLO, CHI, N, NCORE, LOHALF)
    nc = _cached[key]

    from concourse.bass_utils import run_bass_kernel_spmd
    kw = {}
    if os.environ.get("GAT_TRACE", "0") == "1":
        kw = dict(trace=True, tmpdir=os.environ.get("GAT_TRACE_DIR") or None)
    r = run_bass_kernel_spmd(nc, in_maps, list(range(NCORE)), **kw)
    global LAST_EXEC_NS, LAST_RESULT
    LAST_EXEC_NS = r.exec_time_ns
    LAST_RESULT = r
    out = np.concatenate([r.results[k]["outf"] for k in range(NCORE)], axis=0)
    return out.astype(np.float32)


LAST_EXEC_NS = None
LAST_RESULT = None


# revision 11
# speedup vs baseline: 2.1626x; 2.1626x over previous
"""GAT (2-layer, 8-head, mean over heads) Trainium2 Bass kernel, 8-core SPMD.

v3: dst-range sharding with dma_gather-based edge gathers (HW-verified
semantics; the v2 batched indirect-DMA turned out to stream contiguous rows
on HW). Design:

  * Per-node record tables in fp8 (h) with f32 alpha columns bit-cast into
    fp8 slots. Row strides are 256B-multiples (dma_gather constraint):
      hs1:   [h fp8 x512 | a_src f32 x8 @byte 512 | pad]    row 768 B
      hs2:   [h fp8 x256 | a_src f32 x8 @byte 256 | pad]    row 512 B
      adown: [a_dst f32 x8 | pad]                           row 256 B
  * dma_gather uses int16 indices (<32768), so the per-edge src gather is
    split into a lo-rows and hi-rows gather against two row-range views of
    the table; per-block chunk counts for each half are compile-time
    constants (max over cores).
  * Layer-1 tables are built in a PER-CORE PERMUTED row order (own dst range
    first, then the rest in natural order; the host permutes xT's columns),
    which makes "own rows" positional -> the dst-alpha table adown1 and its
    local (int16) dst indices are SPMD-clean.
  * Layer-2 records for the own dst range are computed from the transposed
    layer-1 activations and exchanged with one AllGather (natural row order);
    adown2 is written locally before the AllGather.
  * Per-block (not per-chunk) DVE/ACT processing; selection matrix S built
    with an int8 compare; scatter + denominators via per-chunk matmuls
    accumulating in PSUM.
"""

import os
import ml_dtypes
import numpy as np
from contextlib import ExitStack

N = 50000
E = 800000
H = 8
IN = 256
O1 = 64
O2 = 32
NCORE = 8
P = 128
NEG = 0.2
LOHALF = 32768

ROW1 = 640           # hs1 row, bf16 elements (1280 B)
ROW2 = 384           # hs2 row (768 B)
ROWA = 128           # adown row (256 B)
F1 = H * O1          # 512
F2 = H * O2          # 256

_cached = {}


def _wrap_idx(flat, nidx):
    """dma_gather index layout: [128, nidx//16] int16, idx j at
    [j%16, j//16], replicated to all 8 groups of 16 partitions."""
    a = np.zeros((16, nidx // 16), np.int16)
    a[:, :] = np.asarray(flat, np.int16).reshape(nidx // 16, 16).T
    return np.tile(a, (8, 1))


def _build_meta(edge_index, n, ncore, lohalf):
    ndst = n // ncore
    nblk = (ndst + P - 1) // P
    split = (nblk // 2) * P          # local-row split for the two L2 tables
    src = np.concatenate([edge_index[0], np.arange(n, dtype=np.int64)])
    dst = np.concatenate([edge_index[1], np.arange(n, dtype=np.int64)])

    # per-core sorted edges + per-layer lo/hi split counts
    edges = []          # [core][block] -> (srow1, srow2, dloc) arrays
    nlo = np.zeros((2, ncore, nblk), np.int64)
    nhi = np.zeros((2, ncore, nblk), np.int64)
    for k in range(ncore):
        lo = k * ndst
        m = (dst >= lo) & (dst < lo + ndst)
        s_k = src[m]
        d_k = dst[m] - lo
        o = np.argsort(d_k, kind="stable")
        s_k = s_k[o]
        d_k = d_k[o]
        pos = np.zeros(n, np.int64)
        pos[lo:lo + ndst] = np.arange(ndst)
        others = np.concatenate([np.arange(0, lo), np.arange(lo + ndst, n)])
        pos[others] = ndst + np.arange(n - ndst)
        srow1 = pos[s_k]
        # L2 row ids in the A/B split tables: node g -> core k_g, local i;
        # A rows: k_g*split + i (i < split); B rows: k_g*(ndst-split) + i-split
        kg = s_k // ndst
        ig = s_k % ndst
        inA = ig < split
        srow2 = np.where(inA, kg * split + ig,
                         kg * (ndst - split) + (ig - split))
        blk = d_k // P
        per = []
        for b in range(nblk):
            mb = blk == b
            s1, s2, dl = srow1[mb], srow2[mb], d_k[mb]
            lo1 = s1 < lohalf
            lo2 = inA[mb]
            nlo[0, k, b] = lo1.sum()
            nhi[0, k, b] = (~lo1).sum()
            nlo[1, k, b] = lo2.sum()
            nhi[1, k, b] = (~lo2).sum()
            per.append((s1, s2, dl, lo2))
        edges.append(per)

    cdiv = lambda a, b: -(-a // b)
    CLO = [[int(cdiv(nlo[L, :, b].max(), P)) for b in range(nblk)]
           for L in (0, 1)]
    CHI = [[int(cdiv(nhi[L, :, b].max(), P)) for b in range(nblk)]
           for L in (0, 1)]
    CBb = [[CLO[L][b] + CHI[L][b] for b in range(nblk)] for L in (0, 1)]

    # idx tile column layouts (shared across cores)
    scol = [np.cumsum([0] + [(CLO[L][b] + CHI[L][b]) * 8 for b in range(nblk)])
            for L in (0, 1)]
    acol = [np.cumsum([0] + [CBb[L][b] * 8 for b in range(nblk)])
            for L in (0, 1)]

    CBMAX = max(max(CBb[0]), max(CBb[1]))
    metas = []
    for k in range(ncore):
        per_layer = {}
        for L in (0, 1):
            sidx = np.zeros((P, scol[L][-1]), np.int16)
            lrow = np.full((nblk, CBMAX * P), 200.0, np.float32)
            ld = np.full((P, sum(CBb[L])), -1, np.int8)
            ldoff = np.cumsum([0] + CBb[L])
            for b in range(nblk):
                s1, s2, dl, inA_b = edges[k][b]
                srow = s1 if L == 0 else s2
                isl = (srow < lohalf) if L == 0 else inA_b
                cl, ch = CLO[L][b], CHI[L][b]
                sl = np.zeros(cl * P, np.int64)
                sh = np.zeros(ch * P, np.int64)
                sl[:isl.sum()] = srow[isl]
                sh[:(~isl).sum()] = (srow[~isl] - lohalf) if L == 0 \
                    else srow[~isl]
                dfull = np.full((cl + ch) * P, -1, np.int64)
                dfull[:isl.sum()] = dl[isl]
                dfull[cl * P:cl * P + (~isl).sum()] = dl[~isl]
                c0 = scol[L][b]
                if cl:
                    sidx[:, c0:c0 + cl * 8] = _wrap_idx(sl, cl * P)
                if ch:
                    sidx[:, c0 + cl * 8:c0 + (cl + ch) * 8] = _wrap_idx(
                        sh, ch * P)
                lb = dfull - b * P
                lrow[b, 0:(cl + ch) * P] = np.where(lb >= 0, lb, 200)
                lb2 = lb.copy()
                lb2[dfull < 0] = -1
                ld[:, ldoff[b]:ldoff[b + 1]] = lb2.reshape(cl + ch, P).T
            per_layer[L] = (sidx, lrow, ld)
        metas.append(per_layer)
    key = (tuple(CLO[0]), tuple(CHI[0]), tuple(CLO[1]), tuple(CHI[1]))
    return key, CLO, CHI, metas


def _build_program(CLO, CHI, n, ncore, lohalf):
    import concourse.bacc as bacc
    import concourse.tile as tile
    from concourse import bass, mybir

    f32 = mybir.dt.float32
    bf16 = mybir.dt.bfloat16
    i16 = mybir.dt.int16
    i8 = mybir.dt.int8
    i32 = mybir.dt.int32
    AL = mybir.AluOpType
    AF = mybir.ActivationFunctionType

    ndst = n // ncore
    nblk = (ndst + P - 1) // P
    CBb = [[CLO[L][b] + CHI[L][b] for b in range(nblk)] for L in (0, 1)]
    scol = [np.cumsum([0] + [(CLO[L][b] + CHI[L][b]) * 8
                             for b in range(nblk)]) for L in (0, 1)]
    acol = [np.cumsum([0] + [CBb[L][b] * 8 for b in range(nblk)])
            for L in (0, 1)]
    ldoff = [np.cumsum([0] + CBb[L]) for L in (0, 1)]
    CBMAX = max(max(CBb[0]), max(CBb[1]))

    nc = bacc.Bacc("TRN2", target_bir_lowering=False, debug=False,
                   enable_asserts=True, num_devices=ncore,
                   num_swdge_queues=4)
    xT_d = nc.dram_tensor("xT", [IN, n], bf16, kind="ExternalInput")
    w1h_d = nc.dram_tensor("w1h", [IN, F1], bf16, kind="ExternalInput")
    w1a_d = nc.dram_tensor("w1a", [IN, 16], bf16, kind="ExternalInput")
    w2_d = nc.dram_tensor("w2cat", [O1, 16 + F2], f32, kind="ExternalInput")
    b1_d = nc.dram_tensor("b1rep", [P, O1], f32, kind="ExternalInput")
    b2_d = nc.dram_tensor("b2rep", [P, O2], f32, kind="ExternalInput")
    si1_d = nc.dram_tensor("sidx1", [P, int(scol[0][-1])], i16,
                           kind="ExternalInput")
    lr1_d = nc.dram_tensor("lr1", [nblk, CBMAX * P], bf16,
                           kind="ExternalInput")
    ld1_d = nc.dram_tensor("ld1", [P, int(ldoff[0][-1])], i8,
                           kind="ExternalInput")
    si2_d = nc.dram_tensor("sidx2", [P, int(scol[1][-1])], i16,
                           kind="ExternalInput")
    lr2_d = nc.dram_tensor("lr2", [nblk, CBMAX * P], bf16,
                           kind="ExternalInput")
    ld2_d = nc.dram_tensor("ld2", [P, int(ldoff[1][-1])], i8,
                           kind="ExternalInput")
    outf_d = nc.dram_tensor("outf", [ndst, O2], f32, kind="ExternalOutput")
    hs1 = nc.dram_tensor("hs1", [n, ROW1], bf16)
    ad1t = nc.dram_tensor("ad1t", [ndst, ROWA], bf16)
    ad2t = nc.dram_tensor("ad2t", [ndst, ROWA], bf16)
    split = (nblk // 2) * P
    shr = "Shared" if ncore > 4 else "Local"
    hs2ownA = nc.dram_tensor("hs2ownA", [split, ROW2], bf16)
    hs2ownB = nc.dram_tensor("hs2ownB", [ndst - split, ROW2], bf16)
    hs2A = nc.dram_tensor("hs2A", [ncore * split, ROW2], bf16, addr_space=shr)
    hs2B = nc.dram_tensor("hs2B", [ncore * (ndst - split), ROW2], bf16,
                          addr_space=shr)

    with tile.TileContext(nc) as tc, ExitStack() as ctx:
        cpool = ctx.enter_context(tc.tile_pool(name="const", bufs=1))

        iota_i = cpool.tile([P, P], i32, tag="io_i")
        nc.gpsimd.iota(iota_i[:], pattern=[[1, P]], base=0, channel_multiplier=0)
        iota8 = cpool.tile([P, P], i8, tag="io_8")
        nc.vector.tensor_copy(iota8[:], iota_i[:])
        iotaF = cpool.tile([P, CBMAX * P], i8, tag="iotaF")
        for c in range(CBMAX):
            nc.vector.tensor_copy(iotaF[:, c * P:(c + 1) * P], iota8[:])
        iotac_i = cpool.tile([P, 1], i32, tag="ioc_i")
        nc.gpsimd.iota(iotac_i[:], pattern=[[1, 1]], base=0, channel_multiplier=1)
        iotacf = cpool.tile([P, 1], f32, tag="ioc_f")
        nc.vector.tensor_copy(iotacf[:], iotac_i[:])
        iotaff = cpool.tile([P, P], f32, tag="io_f")
        nc.vector.tensor_copy(iotaff[:], iota_i[:])
        ident = cpool.tile([P, P], f32, tag="ident")
        nc.vector.tensor_scalar(out=ident[:], in0=iotaff[:], scalar1=iotacf[:, 0:1],
                                scalar2=None, op0=AL.is_equal)
        b1s = cpool.tile([P, O1], f32, tag="b1")
        nc.sync.dma_start(out=b1s[:], in_=b1_d.ap()[:, :])
        b2s = cpool.tile([P, O2], f32, tag="b2")
        nc.sync.dma_start(out=b2s[:], in_=b2_d.ap()[:, :])
        xt2sb = cpool.tile([O1, nblk * P], f32, tag="xt2")
        ones_row = cpool.tile([1, P], bf16, tag="ones")
        nc.vector.memset(ones_row[:], 1.0)
        w2s = cpool.tile([O1, 16 + F2], f32, tag="w2")
        nc.sync.dma_start(out=w2s[:], in_=w2_d.ap()[:, :])

        # ---------------- phase A1: layer-1 records (permuted order) --------
        with tc.tile_pool(name="pa_x", bufs=2) as xp, \
             tc.tile_pool(name="pa_w", bufs=1) as wp, \
             tc.tile_pool(name="pa_rec", bufs=3) as rp, \
             tc.tile_pool(name="pa_adr", bufs=3) as arp, \
             tc.tile_pool(name="pa_pm", bufs=3, space="PSUM") as pmp, \
             tc.tile_pool(name="pa_pa", bufs=3, space="PSUM") as pap:
            w1ha = wp.tile([P, F1], bf16, tag="w1ha")
            nc.sync.dma_start(out=w1ha[:], in_=w1h_d.ap()[0:P, :])
            w1hb = wp.tile([P, F1], bf16, tag="w1hb")
            nc.sync.dma_start(out=w1hb[:], in_=w1h_d.ap()[P:IN, :])
            w1aa = wp.tile([P, 16], bf16, tag="w1aa")
            nc.sync.dma_start(out=w1aa[:], in_=w1a_d.ap()[0:P, :])
            w1ab = wp.tile([P, 16], bf16, tag="w1ab")
            nc.sync.dma_start(out=w1ab[:], in_=w1a_d.ap()[P:IN, :])
            CHK = 2048
            for g0 in range(0, n, CHK):
                gw = min(CHK, n - g0)
                xa = xp.tile([P, CHK], bf16, tag="xa")
                nc.sync.dma_start(out=xa[:, :gw], in_=xT_d.ap()[0:P, g0:g0 + gw])
                xb = xp.tile([P, CHK], bf16, tag="xb")
                nc.sync.dma_start(out=xb[:, :gw], in_=xT_d.ap()[P:IN, g0:g0 + gw])
                for off in range(0, gw, P):
                    m = min(P, gw - off)
                    row0 = g0 + off
                    psh = pmp.tile([P, F1], f32, tag="psh")
                    nc.tensor.matmul(psh[:m, :], lhsT=xa[:, off:off + m],
                                     rhs=w1ha[:, :], start=True, stop=False)
                    nc.tensor.matmul(psh[:m, :], lhsT=xb[:, off:off + m],
                                     rhs=w1hb[:, :], start=False, stop=True)
                    psa = pap.tile([P, 16], f32, tag="psa")
                    nc.tensor.matmul(psa[:m, :], lhsT=xa[:, off:off + m],
                                     rhs=w1aa[:, :], start=True, stop=False)
                    nc.tensor.matmul(psa[:m, :], lhsT=xb[:, off:off + m],
                                     rhs=w1ab[:, :], start=False, stop=True)
                    rec = rp.tile([P, F1 + 16], bf16, tag="rec")
                    if (row0 // P) % 2 == 0:
                        nc.scalar.activation(out=rec[:m, 0:F1], in_=psh[:m, :],
                                             func=AF.Copy)
                    else:
                        nc.vector.tensor_copy(rec[:m, 0:F1], psh[:m, :])
                    recf = rec[:].bitcast(f32)
                    nc.vector.tensor_copy(recf[:m, 256:256 + H], psa[:m, 0:H])
                    nc.sync.dma_start(out=hs1.ap()[row0:row0 + m, 0:F1 + 16],
                                      in_=rec[:m, :])
                    if row0 < ndst:
                        mm = min(m, ndst - row0)
                        adr = arp.tile([P, 16], bf16, tag="adr")
                        adrf = adr[:].bitcast(f32)
                        nc.vector.tensor_copy(adrf[:mm, 0:H], psa[:mm, H:2 * H])
                        nc.sync.dma_start(out=ad1t.ap()[row0:row0 + mm, 0:16],
                                          in_=adr[:mm, :])

        # ---------------- edge phase ----------------------------------------
        qctr = [0]

        def edge_phase(layer):
            L = layer - 1
            if layer == 1:
                ROW, Fh, F, adt = ROW1, O1, F1, ad1t
                si_d_, lr_d_, ld_d_ = si1_d, lr1_d, ld1_d
                ASF = 256            # f32 index of a_src in row
                viewlo = hs1.ap()[0:lohalf, :]
                viewhi = hs1.ap()[lohalf:n, :]
            else:
                ROW, Fh, F, adt = ROW2, O2, F2, ad2t
                si_d_, lr_d_, ld_d_ = si2_d, lr2_d, ld2_d
                ASF = 128
                viewlo = hs2A.ap()[:, :]
                viewhi = hs2B.ap()[:, :]
            RF = ROW // 4            # f32 elements per row
            with tc.tile_pool(name=f"ep{layer}_idx", bufs=1) as idxp, \
                 tc.tile_pool(name=f"ep{layer}_rec", bufs=2) as recp, \
                 tc.tile_pool(name=f"ep{layer}_adv", bufs=2) as advp, \
                 tc.tile_pool(name=f"ep{layer}_S", bufs=2) as sp, \
                 tc.tile_pool(name=f"ep{layer}_sm", bufs=2) as smp, \
                 tc.tile_pool(name=f"ep{layer}_msg", bufs=2) as msgp, \
                 tc.tile_pool(name=f"ep{layer}_epi", bufs=2) as epi, \
                 tc.tile_pool(name=f"ep{layer}_lr", bufs=2) as lrp, \
                 tc.tile_pool(name=f"ep{layer}_St", bufs=2) as stp, \
                 tc.tile_pool(name=f"ep{layer}_out", bufs=2, space="PSUM") as outp_, \
                 tc.tile_pool(name=f"ep{layer}_den", bufs=2, space="PSUM") as denp, \
                 tc.tile_pool(name=f"ep{layer}_tr", bufs=1, space="PSUM") as trp, \
                 tc.tile_pool(name=f"ep{layer}_ade", bufs=2, space="PSUM") as adep, \
                 tc.tile_pool(name=f"ep{layer}_ps2", bufs=1, space="PSUM") as pp2:
                si_sb = idxp.tile([P, int(scol[L][-1])], i16, tag="si")
                nc.sync.dma_start(out=si_sb[:], in_=si_d_.ap()[:, :])
                ld_sb = idxp.tile([P, int(ldoff[L][-1])], i8, tag="ld")
                nc.sync.dma_start(out=ld_sb[:], in_=ld_d_.ap()[:, :])
                for b in range(nblk):
                    bbase = b * P
                    bm = min(P, ndst - bbase)
                    CL, CH = CLO[L][b], CHI[L][b]
                    CB = CL + CH
                    lo0 = int(ldoff[L][b])
                    rec = recp.tile([P, CB * ROW], bf16, tag="rec")
                    s0 = int(scol[L][b])
                    if CL:
                        nc.gpsimd.dma_gather(
                            rec[:, 0:CL * ROW].rearrange("p (c r) -> p c r",
                                                         r=ROW),
                            viewlo,
                            si_sb[:, s0:s0 + CL * 8],
                            CL * P, CL * P, ROW, single_packet=False,
                            queue_num=qctr[0] % 4)
                        qctr[0] += 1
                    if CH:
                        nc.gpsimd.dma_gather(
                            rec[:, CL * ROW:CB * ROW].rearrange(
                                "p (c r) -> p c r", r=ROW),
                            viewhi,
                            si_sb[:, s0 + CL * 8:s0 + CB * 8],
                            CH * P, CH * P, ROW, single_packet=False,
                            queue_num=qctr[0] % 4)
                        qctr[0] += 1
                    # dst alphas: contiguous local rows -> plain DMA + bcast
                    adv = advp.tile([P, 16], bf16, tag="adv")
                    nc.sync.dma_start(out=adv[:bm, :],
                                      in_=adt.ap()[bbase:bbase + bm, 0:16])
                    adb = advp.tile([P, H], bf16, tag="adb")
                    nc.scalar.activation(out=adb[:], in_=adv[:].bitcast(f32),
                                         func=AF.Copy)
                    lrow = lrp.tile([P, CBMAX * P], bf16, tag="lrow")
                    nc.sync.dma_start(
                        out=lrow[:, 0:CB * P],
                        in_=lr_d_.ap()[b:b + 1, 0:CB * P]
                            .partition_broadcast(P))
                    St = stp.tile([P, CB * P], bf16, tag="St")
                    nc.vector.tensor_scalar(out=St[:, 0:CB * P],
                                            in0=lrow[:, 0:CB * P],
                                            scalar1=iotacf[:, 0:1],
                                            scalar2=None, op0=AL.is_equal)
                    ade = adep.tile([P, CB * H], f32, tag="ade")
                    for c in range(CB):
                        nc.tensor.matmul(ade[:, c * H:(c + 1) * H],
                                         lhsT=St[:, c * P:(c + 1) * P],
                                         rhs=adb[:], start=True, stop=True)
                    S = sp.tile([P, CB * P], bf16, tag="S")
                    nc.vector.tensor_tensor(
                        out=S[:].rearrange("p (c j) -> p c j", c=CB),
                        in0=iotaF[:, 0:CB * P].rearrange("p (c j) -> p c j",
                                                         c=CB),
                        in1=ld_sb[:, lo0:lo0 + CB].to_broadcast([P, CB, P]),
                        op=AL.is_equal)
                    recf = rec[:].bitcast(f32)
                    et = smp.tile([P, CB * H], f32, tag="et")
                    nc.vector.tensor_tensor(
                        out=et[:].rearrange("p (c h) -> p c h", c=CB),
                        in0=recf.rearrange("p (c r) -> p c r", c=CB)
                            [:, :, ASF:ASF + H],
                        in1=ade[:].rearrange("p (c h) -> p c h", c=CB),
                        op=AL.add)
                    lt = smp.tile([P, CB * H], f32, tag="lt")
                    nc.vector.scalar_tensor_tensor(out=lt[:], in0=et[:],
                                                   scalar=NEG, in1=et[:],
                                                   op0=AL.mult, op1=AL.max)
                    ex = smp.tile([P, CB * H], f32, tag="ex")
                    nc.scalar.activation(out=ex[:], in_=lt[:], func=AF.Exp)
                    exb = smp.tile([P, CB * H], bf16, tag="exb")
                    nc.scalar.activation(out=exb[:], in_=ex[:], func=AF.Copy)
                    msgb = msgp.tile([P, CB * F], bf16, tag="msgb")
                    nc.vector.tensor_tensor(
                        out=msgb[:].rearrange("p (c h f) -> p c h f",
                                              c=CB, h=H),
                        in0=rec[:].rearrange("p (c r) -> p c r", c=CB)
                            [:, :, 0:F].rearrange("p c (h f) -> p c h f", h=H),
                        in1=ex[:].rearrange("p (c h) -> p c h", c=CB)
                            .to_broadcast([P, CB, H, Fh]),
                        op=AL.mult)
                    outp = outp_.tile([P, F], f32, tag="out")
                    den = denp.tile([P, H], f32, tag="den")
                    for c in range(CB):
                        nc.tensor.matmul(outp[:], lhsT=S[:, c * P:(c + 1) * P],
                                         rhs=msgb[:, c * F:(c + 1) * F],
                                         start=(c == 0), stop=(c == CB - 1))
                        nc.tensor.matmul(den[:], lhsT=S[:, c * P:(c + 1) * P],
                                         rhs=exb[:, c * H:(c + 1) * H],
                                         start=(c == 0), stop=(c == CB - 1))
                    r = epi.tile([P, H], f32, tag="r")
                    nc.vector.tensor_scalar(out=r[:], in0=den[:], scalar1=1e-16,
                                            scalar2=None, op0=AL.add)
                    nc.vector.reciprocal(r[:], r[:])
                    nc.vector.tensor_scalar(out=r[:], in0=r[:], scalar1=1.0 / H,
                                            scalar2=None, op0=AL.mult)
                    tmp0 = epi.tile([P, F], f32, tag="tmp0")
                    nc.scalar.activation(out=tmp0[:], in_=outp[:], func=AF.Copy)
                    tmp = epi.tile([P, F], f32, tag="tmp")
                    nc.vector.tensor_tensor(
                        out=tmp[:].rearrange("p (h f) -> p h f", h=H),
                        in0=tmp0[:].rearrange("p (h f) -> p h f", h=H),
                        in1=r[:].to_broadcast([P, H, Fh]),
                        op=AL.mult)
                    acc = epi.tile([P, Fh], f32, tag="acc")
                    nc.vector.tensor_reduce(
                        out=acc[:], in_=tmp[:].rearrange("p (h f) -> p f h",
                                                         h=H),
                        axis=mybir.AxisListType.X, op=AL.add)
                    bs = b1s if layer == 1 else b2s
                    nc.vector.tensor_tensor(out=acc[:], in0=acc[:],
                                            in1=bs[:, 0:Fh], op=AL.add)
                    if layer == 1:
                        x2t = epi.tile([P, O1], f32, tag="x2")
                        nc.vector.tensor_scalar(out=x2t[:], in0=acc[:],
                                                scalar1=0.0, scalar2=None,
                                                op0=AL.max)
                        tr = trp.tile([O1, P], f32, tag="tr")
                        nc.tensor.transpose(out=tr[:], in_=x2t[:],
                                            identity=ident[:])
                        nc.vector.tensor_copy(xt2sb[:, bbase:bbase + P], tr[:])
                        ps2 = pp2.tile([P, 16 + F2], f32, tag="ps2")
                        nc.tensor.matmul(ps2[:bm, :],
                                         lhsT=xt2sb[:, bbase:bbase + bm],
                                         rhs=w2s[:, :], start=True, stop=True)
                        rec2 = epi.tile([P, F2 + 16], bf16, tag="rec2")
                        nc.scalar.activation(out=rec2[:bm, 0:F2],
                                             in_=ps2[:bm, 16:16 + F2],
                                             func=AF.Copy)
                        rec2f = rec2[:].bitcast(f32)
                        nc.vector.tensor_copy(rec2f[:bm, 128:128 + H],
                                              ps2[:bm, 0:H])
                        if b < nblk // 2:
                            nc.sync.dma_start(
                                out=hs2ownA.ap()[bbase:bbase + bm, 0:F2 + 16],
                                in_=rec2[:bm, :])
                        else:
                            nc.sync.dma_start(
                                out=hs2ownB.ap()[bbase - split:
                                                 bbase - split + bm,
                                                 0:F2 + 16],
                                in_=rec2[:bm, :])
                        adr2 = epi.tile([P, 16], bf16, tag="adr2")
                        adr2f = adr2[:].bitcast(f32)
                        nc.vector.tensor_copy(adr2f[:bm, 0:H],
                                              ps2[:bm, H:2 * H])
                        nc.sync.dma_start(out=ad2t.ap()[bbase:bbase + bm, 0:16],
                                          in_=adr2[:bm, :])
                        if b == nblk // 2 - 1:
                            if ncore > 1:
                                nc.gpsimd.collective_compute(
                                    "AllGather", mybir.AluOpType.bypass,
                                    replica_groups=[list(range(ncore))],
                                    ins=[hs2ownA.ap().opt()],
                                    outs=[hs2A.ap().opt()])
                            else:
                                nc.sync.dma_start(out=hs2A.ap()[:, :],
                                                  in_=hs2ownA.ap()[:, :])
                    else:
                        f = epi.tile([P, O2], f32, tag="f")
                        nc.vector.tensor_scalar(out=f[:], in0=acc[:],
                                                scalar1=0.0, scalar2=None,
                                                op0=AL.max)
                        nmx = epi.tile([P, 1], f32, tag="nmx")
                        nc.vector.tensor_reduce(out=nmx[:], in_=f[:],
                                                axis=mybir.AxisListType.X,
                                                op=AL.max, negate=True)
                        ef = epi.tile([P, O2], f32, tag="ef")
                        nc.scalar.activation(out=ef[:], in_=f[:], func=AF.Exp,
                                             bias=nmx[:, 0:1])
                        sm = epi.tile([P, 1], f32, tag="sm")
                        nc.vector.tensor_reduce(out=sm[:], in_=ef[:],
                                                axis=mybir.AxisListType.X,
                                                op=AL.add)
                        rs = epi.tile([P, 1], f32, tag="rs")
                        nc.vector.reciprocal(rs[:], sm[:])
                        nc.vector.tensor_scalar(out=ef[:], in0=ef[:],
                                                scalar1=rs[:, 0:1], scalar2=None,
                                                op0=AL.mult)
                        nc.sync.dma_start(out=outf_d.ap()[bbase:bbase + bm, :],
                                          in_=ef[:bm, :])

        edge_phase(1)

        if ncore > 1:
            nc.gpsimd.collective_compute(
                "AllGather", mybir.AluOpType.bypass,
                replica_groups=[list(range(ncore))],
                ins=[hs2ownB.ap().opt()], outs=[hs2B.ap().opt()])
        else:
            nc.sync.dma_start(out=hs2B.ap()[:, :], in_=hs2ownB.ap()[:, :])

        edge_phase(2)

    nc.compile()
    # Align each gather's SWDGE queue with its tile-assigned DMASW sem lane
    # (a sem lane is locked to one queue; lanes are assigned in scheduled
    # order, so the queue must be derived, not chosen up front).
    import re as _re
    for fn in nc.m.functions:
        for bb in fn.blocks:
            for inst in bb.instructions:
                if type(inst).__name__ == "InstDMAGatherAnt":
                    si = inst.sync_info
                    for u in (si.on_update if si is not None else []):
                        mm = _re.match(r"DMASW(\d+)_", u.ant_name or "")
                        if mm:
                            inst.queue_num = int(mm.group(1)) % 4
                            break
    return nc


def _prep_inputs(x, edge_index, W1, a_src1, a_dst1, b1, W2, a_src2, a_dst2, b2,
                 n, ncore, lohalf):
    ndst = n // ncore
    x = np.asarray(x, np.float32)
    W1 = np.asarray(W1, np.float32)
    W2 = np.asarray(W2, np.float32)
    As1 = np.einsum("hf,hfc->ch", np.asarray(a_src1, np.float32),
                    W1.reshape(H, O1, IN)).astype(np.float32)
    Ad1 = np.einsum("hf,hfc->ch", np.asarray(a_dst1, np.float32),
                    W1.reshape(H, O1, IN)).astype(np.float32)
    w1h = np.ascontiguousarray(W1.T)
    w1a = np.ascontiguousarray(np.concatenate([As1, Ad1], axis=1))
    As2 = np.einsum("hf,hfc->ch", np.asarray(a_src2, np.float32),
                    W2.reshape(H, O2, O1)).astype(np.float32)
    Ad2 = np.einsum("hf,hfc->ch", np.asarray(a_dst2, np.float32),
                    W2.reshape(H, O2, O1)).astype(np.float32)
    w2cat = np.ascontiguousarray(
        np.concatenate([As2, Ad2, W2.T], axis=1)).astype(np.float32)
    b1rep = np.ascontiguousarray(
        np.tile(np.asarray(b1, np.float32)[None, :], (P, 1)))
    b2rep = np.ascontiguousarray(
        np.tile(np.asarray(b2, np.float32)[None, :], (P, 1)))

    key, CLO, CHI, metas = _build_meta(np.asarray(edge_index), n, ncore,
                                       lohalf)
    xT = x.T
    in_maps = []
    for k in range(ncore):
        lo = k * ndst
        perm = np.concatenate([np.arange(lo, lo + ndst),
                               np.arange(0, lo), np.arange(lo + ndst, n)])
        sidx1, lrow1, ld1 = metas[k][0]
        sidx2, lrow2, ld2 = metas[k][1]
        in_maps.append({
            "xT": np.ascontiguousarray(xT[:, perm]).astype(ml_dtypes.bfloat16),
            "w1h": w1h.astype(ml_dtypes.bfloat16),
            "w1a": w1a.astype(ml_dtypes.bfloat16),
            "w2cat": w2cat,
            "b1rep": b1rep, "b2rep": b2rep,
            "sidx1": sidx1, "lr1": lrow1.astype(ml_dtypes.bfloat16),
            "ld1": ld1,
            "sidx2": sidx2, "lr2": lrow2.astype(ml_dtypes.bfloat16),
            "ld2": ld2,
        })
    return key, CLO, CHI, in_maps


def kernel(x, edge_index, W1, a_src1, a_dst1, b1, W2, a_src2, a_dst2, b2):
    key, CLO, CHI, in_maps = _prep_inputs(
        x, edge_index, W1, a_src1, a_dst1, b1, W2, a_src2, a_dst2, b2,
        N, NCORE, LOHALF)
    if key not in _cached:
        _cached[key] = _build_program(CLO, CHI, N, NCORE, LOHALF)
    nc = _cached[key]

    from concourse.bass_utils import run_bass_kernel_spmd
    kw = {}
    if os.environ.get("GAT_TRACE", "0") == "1":
        kw = dict(trace=True, tmpdir=os.environ.get("GAT_TRACE_DIR") or None)
    r = run_bass_kernel_spmd(nc, in_maps, list(range(NCORE)), **kw)
    global LAST_EXEC_NS, LAST_RESULT
    LAST_EXEC_NS = r.exec_time_ns
    LAST_RESULT = r
    out = np.concatenate([r.results[k]["outf"] for k in range(NCORE)], axis=0)
    return out.astype(np.float32)


LAST_EXEC_NS = None
LAST_RESULT = None


# revision 13
# speedup vs baseline: 2.4938x; 1.1531x over previous
"""GAT (2-layer, 8-head, mean over heads) Trainium2 Bass kernel, 8-core SPMD.

v3: dst-range sharding with dma_gather-based edge gathers (HW-verified
semantics; the v2 batched indirect-DMA turned out to stream contiguous rows
on HW). Design:

  * Per-node record tables in fp8 (h) with f32 alpha columns bit-cast into
    fp8 slots. Row strides are 256B-multiples (dma_gather constraint):
      hs1:   [h fp8 x512 | a_src f32 x8 @byte 512 | pad]    row 768 B
      hs2:   [h fp8 x256 | a_src f32 x8 @byte 256 | pad]    row 512 B
      adown: [a_dst f32 x8 | pad]                           row 256 B
  * dma_gather uses int16 indices (<32768), so the per-edge src gather is
    split into a lo-rows and hi-rows gather against two row-range views of
    the table; per-block chunk counts for each half are compile-time
    constants (max over cores).
  * Layer-1 tables are built in a PER-CORE PERMUTED row order (own dst range
    first, then the rest in natural order; the host permutes xT's columns),
    which makes "own rows" positional -> the dst-alpha table adown1 and its
    local (int16) dst indices are SPMD-clean.
  * Layer-2 records for the own dst range are computed from the transposed
    layer-1 activations and exchanged with one AllGather (natural row order);
    adown2 is written locally before the AllGather.
  * Per-block (not per-chunk) DVE/ACT processing; selection matrix S built
    with an int8 compare; scatter + denominators via per-chunk matmuls
    accumulating in PSUM.
"""

import os
import ml_dtypes
import numpy as np
from contextlib import ExitStack

N = 50000
E = 800000
H = 8
IN = 256
O1 = 64
O2 = 32
NCORE = 8
P = 128
NEG = 0.2
LOHALF = 32768

ROW1 = 640           # hs1 row, bf16 elements (1280 B)
ROW2 = 384           # hs2 row (768 B)
ROWA = 128           # adown row (256 B)
F1 = H * O1          # 512
F2 = H * O2          # 256

_cached = {}


def _wrap_idx(flat, nidx):
    """dma_gather index layout: [128, nidx//16] int16, idx j at
    [j%16, j//16], replicated to all 8 groups of 16 partitions."""
    a = np.zeros((16, nidx // 16), np.int16)
    a[:, :] = np.asarray(flat, np.int16).reshape(nidx // 16, 16).T
    return np.tile(a, (8, 1))


def _build_meta(edge_index, n, ncore, lohalf):
    ndst = n // ncore
    nblk = (ndst + P - 1) // P
    split = (nblk // 2) * P          # local-row split for the two L2 tables
    src = np.concatenate([edge_index[0], np.arange(n, dtype=np.int64)])
    dst = np.concatenate([edge_index[1], np.arange(n, dtype=np.int64)])

    # per-core sorted edges + per-layer lo/hi split counts
    edges = []          # [core][block] -> (srow1, srow2, dloc) arrays
    nlo = np.zeros((2, ncore, nblk), np.int64)
    nhi = np.zeros((2, ncore, nblk), np.int64)
    for k in range(ncore):
        lo = k * ndst
        m = (dst >= lo) & (dst < lo + ndst)
        s_k = src[m]
        d_k = dst[m] - lo
        o = np.argsort(d_k, kind="stable")
        s_k = s_k[o]
        d_k = d_k[o]
        pos = np.zeros(n, np.int64)
        pos[lo:lo + ndst] = np.arange(ndst)
        others = np.concatenate([np.arange(0, lo), np.arange(lo + ndst, n)])
        pos[others] = ndst + np.arange(n - ndst)
        srow1 = pos[s_k]
        # L2 row ids in the A/B split tables: node g -> core k_g, local i;
        # A rows: k_g*split + i (i < split); B rows: k_g*(ndst-split) + i-split
        kg = s_k // ndst
        ig = s_k % ndst
        inA = ig < split
        srow2 = np.where(inA, kg * split + ig,
                         kg * (ndst - split) + (ig - split))
        blk = d_k // P
        per = []
        for b in range(nblk):
            mb = blk == b
            s1, s2, dl = srow1[mb], srow2[mb], d_k[mb]
            lo1 = s1 < lohalf
            lo2 = inA[mb]
            nlo[0, k, b] = lo1.sum()
            nhi[0, k, b] = (~lo1).sum()
            nlo[1, k, b] = lo2.sum()
            nhi[1, k, b] = (~lo2).sum()
            per.append((s1, s2, dl, lo2))
        edges.append(per)

    cdiv = lambda a, b: -(-a // b)
    CLO = [[int(cdiv(nlo[L, :, b].max(), P)) for b in range(nblk)]
           for L in (0, 1)]
    CHI = [[int(cdiv(nhi[L, :, b].max(), P)) for b in range(nblk)]
           for L in (0, 1)]
    CBb = [[CLO[L][b] + CHI[L][b] for b in range(nblk)] for L in (0, 1)]

    # idx tile column layouts (shared across cores)
    scol = [np.cumsum([0] + [(CLO[L][b] + CHI[L][b]) * 8 for b in range(nblk)])
            for L in (0, 1)]
    acol = [np.cumsum([0] + [CBb[L][b] * 8 for b in range(nblk)])
            for L in (0, 1)]

    CBMAX = max(max(CBb[0]), max(CBb[1]))
    metas = []
    for k in range(ncore):
        per_layer = {}
        for L in (0, 1):
            sidx = np.zeros((P, scol[L][-1]), np.int16)
            lrow = np.full((nblk, CBMAX * P), 200.0, np.float32)
            ld = np.full((P, sum(CBb[L])), -1, np.int8)
            ldoff = np.cumsum([0] + CBb[L])
            for b in range(nblk):
                s1, s2, dl, inA_b = edges[k][b]
                srow = s1 if L == 0 else s2
                isl = (srow < lohalf) if L == 0 else inA_b
                cl, ch = CLO[L][b], CHI[L][b]
                sl = np.zeros(cl * P, np.int64)
                sh = np.zeros(ch * P, np.int64)
                sl[:isl.sum()] = srow[isl]
                sh[:(~isl).sum()] = (srow[~isl] - lohalf) if L == 0 \
                    else srow[~isl]
                dfull = np.full((cl + ch) * P, -1, np.int64)
                dfull[:isl.sum()] = dl[isl]
                dfull[cl * P:cl * P + (~isl).sum()] = dl[~isl]
                c0 = scol[L][b]
                if cl:
                    sidx[:, c0:c0 + cl * 8] = _wrap_idx(sl, cl * P)
                if ch:
                    sidx[:, c0 + cl * 8:c0 + (cl + ch) * 8] = _wrap_idx(
                        sh, ch * P)
                lb = dfull - b * P
                lrow[b, 0:(cl + ch) * P] = np.where(lb >= 0, lb, 200)
                lb2 = lb.copy()
                lb2[dfull < 0] = -1
                ld[:, ldoff[b]:ldoff[b + 1]] = lb2.reshape(cl + ch, P).T
            per_layer[L] = (sidx, lrow, ld)
        metas.append(per_layer)
    key = (tuple(CLO[0]), tuple(CHI[0]), tuple(CLO[1]), tuple(CHI[1]))
    return key, CLO, CHI, metas


def _build_program(CLO, CHI, n, ncore, lohalf):
    import concourse.bacc as bacc
    import concourse.tile as tile
    from concourse import bass, mybir

    f32 = mybir.dt.float32
    bf16 = mybir.dt.bfloat16
    i16 = mybir.dt.int16
    i8 = mybir.dt.int8
    i32 = mybir.dt.int32
    AL = mybir.AluOpType
    AF = mybir.ActivationFunctionType

    ndst = n // ncore
    nblk = (ndst + P - 1) // P
    CBb = [[CLO[L][b] + CHI[L][b] for b in range(nblk)] for L in (0, 1)]
    scol = [np.cumsum([0] + [(CLO[L][b] + CHI[L][b]) * 8
                             for b in range(nblk)]) for L in (0, 1)]
    acol = [np.cumsum([0] + [CBb[L][b] * 8 for b in range(nblk)])
            for L in (0, 1)]
    ldoff = [np.cumsum([0] + CBb[L]) for L in (0, 1)]
    CBMAX = max(max(CBb[0]), max(CBb[1]))

    nc = bacc.Bacc("TRN2", target_bir_lowering=False, debug=False,
                   enable_asserts=True, num_devices=ncore,
                   num_swdge_queues=4)
    xT_d = nc.dram_tensor("xT", [IN, n], bf16, kind="ExternalInput")
    w1h_d = nc.dram_tensor("w1h", [IN, F1], bf16, kind="ExternalInput")
    w1a_d = nc.dram_tensor("w1a", [IN, 16], bf16, kind="ExternalInput")
    w2_d = nc.dram_tensor("w2cat", [O1, 16 + F2], f32, kind="ExternalInput")
    b1_d = nc.dram_tensor("b1rep", [P, O1], f32, kind="ExternalInput")
    b2_d = nc.dram_tensor("b2rep", [P, O2], f32, kind="ExternalInput")
    si1_d = nc.dram_tensor("sidx1", [P, int(scol[0][-1])], i16,
                           kind="ExternalInput")
    lr1_d = nc.dram_tensor("lr1", [nblk, CBMAX * P], bf16,
                           kind="ExternalInput")
    ld1_d = nc.dram_tensor("ld1", [P, int(ldoff[0][-1])], i8,
                           kind="ExternalInput")
    si2_d = nc.dram_tensor("sidx2", [P, int(scol[1][-1])], i16,
                           kind="ExternalInput")
    lr2_d = nc.dram_tensor("lr2", [nblk, CBMAX * P], bf16,
                           kind="ExternalInput")
    ld2_d = nc.dram_tensor("ld2", [P, int(ldoff[1][-1])], i8,
                           kind="ExternalInput")
    outf_d = nc.dram_tensor("outf", [ndst, O2], f32, kind="ExternalOutput")
    hs1 = nc.dram_tensor("hs1", [n, ROW1], bf16)
    ad1t = nc.dram_tensor("ad1t", [ndst, ROWA], bf16)
    ad2t = nc.dram_tensor("ad2t", [ndst, ROWA], bf16)
    split = (nblk // 2) * P
    shr = "Shared" if ncore > 4 else "Local"
    hs2ownA = nc.dram_tensor("hs2ownA", [split, ROW2], bf16)
    hs2ownB = nc.dram_tensor("hs2ownB", [ndst - split, ROW2], bf16)
    hs2A = nc.dram_tensor("hs2A", [ncore * split, ROW2], bf16, addr_space=shr)
    hs2B = nc.dram_tensor("hs2B", [ncore * (ndst - split), ROW2], bf16,
                          addr_space=shr)

    with tile.TileContext(nc) as tc, ExitStack() as ctx:
        cpool = ctx.enter_context(tc.tile_pool(name="const", bufs=1))

        iota_i = cpool.tile([P, P], i32, tag="io_i")
        nc.gpsimd.iota(iota_i[:], pattern=[[1, P]], base=0, channel_multiplier=0)
        iota8 = cpool.tile([P, P], i8, tag="io_8")
        nc.vector.tensor_copy(iota8[:], iota_i[:])
        iotaF = cpool.tile([P, CBMAX * P], i8, tag="iotaF")
        for c in range(CBMAX):
            nc.vector.tensor_copy(iotaF[:, c * P:(c + 1) * P], iota8[:])
        iotac_i = cpool.tile([P, 1], i32, tag="ioc_i")
        nc.gpsimd.iota(iotac_i[:], pattern=[[1, 1]], base=0, channel_multiplier=1)
        iotacf = cpool.tile([P, 1], f32, tag="ioc_f")
        nc.vector.tensor_copy(iotacf[:], iotac_i[:])
        iotaff = cpool.tile([P, P], f32, tag="io_f")
        nc.vector.tensor_copy(iotaff[:], iota_i[:])
        ident = cpool.tile([P, P], f32, tag="ident")
        nc.vector.tensor_scalar(out=ident[:], in0=iotaff[:], scalar1=iotacf[:, 0:1],
                                scalar2=None, op0=AL.is_equal)
        b1s = cpool.tile([P, O1], f32, tag="b1")
        nc.sync.dma_start(out=b1s[:], in_=b1_d.ap()[:, :])
        b2s = cpool.tile([P, O2], f32, tag="b2")
        nc.sync.dma_start(out=b2s[:], in_=b2_d.ap()[:, :])
        xt2sb = cpool.tile([O1, nblk * P], f32, tag="xt2")
        ones_row = cpool.tile([1, P], bf16, tag="ones")
        nc.vector.memset(ones_row[:], 1.0)
        w2s = cpool.tile([O1, 16 + F2], f32, tag="w2")
        nc.sync.dma_start(out=w2s[:], in_=w2_d.ap()[:, :])

        # ---------------- phase A1: layer-1 records (permuted order) --------
        with tc.tile_pool(name="pa_x", bufs=2) as xp, \
             tc.tile_pool(name="pa_w", bufs=1) as wp, \
             tc.tile_pool(name="pa_rec", bufs=3) as rp, \
             tc.tile_pool(name="pa_adr", bufs=3) as arp, \
             tc.tile_pool(name="pa_pm", bufs=3, space="PSUM") as pmp, \
             tc.tile_pool(name="pa_pa", bufs=3, space="PSUM") as pap:
            w1ha = wp.tile([P, F1], bf16, tag="w1ha")
            nc.sync.dma_start(out=w1ha[:], in_=w1h_d.ap()[0:P, :])
            w1hb = wp.tile([P, F1], bf16, tag="w1hb")
            nc.sync.dma_start(out=w1hb[:], in_=w1h_d.ap()[P:IN, :])
            w1aa = wp.tile([P, 16], bf16, tag="w1aa")
            nc.sync.dma_start(out=w1aa[:], in_=w1a_d.ap()[0:P, :])
            w1ab = wp.tile([P, 16], bf16, tag="w1ab")
            nc.sync.dma_start(out=w1ab[:], in_=w1a_d.ap()[P:IN, :])
            CHK = 2048
            for g0 in range(0, n, CHK):
                gw = min(CHK, n - g0)
                xa = xp.tile([P, CHK], bf16, tag="xa")
                nc.sync.dma_start(out=xa[:, :gw], in_=xT_d.ap()[0:P, g0:g0 + gw])
                xb = xp.tile([P, CHK], bf16, tag="xb")
                nc.sync.dma_start(out=xb[:, :gw], in_=xT_d.ap()[P:IN, g0:g0 + gw])
                for off in range(0, gw, P):
                    m = min(P, gw - off)
                    row0 = g0 + off
                    psh = pmp.tile([P, F1], f32, tag="psh")
                    nc.tensor.matmul(psh[:m, :], lhsT=xa[:, off:off + m],
                                     rhs=w1ha[:, :], start=True, stop=False)
                    nc.tensor.matmul(psh[:m, :], lhsT=xb[:, off:off + m],
                                     rhs=w1hb[:, :], start=False, stop=True)
                    psa = pap.tile([P, 16], f32, tag="psa")
                    nc.tensor.matmul(psa[:m, :], lhsT=xa[:, off:off + m],
                                     rhs=w1aa[:, :], start=True, stop=False)
                    nc.tensor.matmul(psa[:m, :], lhsT=xb[:, off:off + m],
                                     rhs=w1ab[:, :], start=False, stop=True)
                    rec = rp.tile([P, F1 + 16], bf16, tag="rec")
                    nc.scalar.activation(out=rec[:m, 0:F1], in_=psh[:m, :],
                                         func=AF.Copy)
                    recf = rec[:].bitcast(f32)
                    nc.vector.tensor_copy(recf[:m, 256:256 + H], psa[:m, 0:H])
                    nc.sync.dma_start(out=hs1.ap()[row0:row0 + m, 0:F1 + 16],
                                      in_=rec[:m, :])
                    if row0 < ndst:
                        mm = min(m, ndst - row0)
                        adr = arp.tile([P, 16], bf16, tag="adr")
                        adrf = adr[:].bitcast(f32)
                        nc.vector.tensor_copy(adrf[:mm, 0:H], psa[:mm, H:2 * H])
                        nc.sync.dma_start(out=ad1t.ap()[row0:row0 + mm, 0:16],
                                          in_=adr[:mm, :])

        # ---------------- edge phase ----------------------------------------
        qctr = [0]

        def edge_phase(layer):
            L = layer - 1
            if layer == 1:
                ROW, Fh, F, adt = ROW1, O1, F1, ad1t
                si_d_, lr_d_, ld_d_ = si1_d, lr1_d, ld1_d
                ASF = 256            # f32 index of a_src in row
                viewlo = hs1.ap()[0:lohalf, :]
                viewhi = hs1.ap()[lohalf:n, :]
            else:
                ROW, Fh, F, adt = ROW2, O2, F2, ad2t
                si_d_, lr_d_, ld_d_ = si2_d, lr2_d, ld2_d
                ASF = 128
                viewlo = hs2A.ap()[:, :]
                viewhi = hs2B.ap()[:, :]
            RF = ROW // 4            # f32 elements per row
            with tc.tile_pool(name=f"ep{layer}_idx", bufs=1) as idxp, \
                 tc.tile_pool(name=f"ep{layer}_rec", bufs=2) as recp, \
                 tc.tile_pool(name=f"ep{layer}_adv", bufs=3) as advp, \
                 tc.tile_pool(name=f"ep{layer}_S", bufs=3) as sp, \
                 tc.tile_pool(name=f"ep{layer}_sm", bufs=3) as smp, \
                 tc.tile_pool(name=f"ep{layer}_msg", bufs=2) as msgp, \
                 tc.tile_pool(name=f"ep{layer}_epi", bufs=2) as epi, \
                 tc.tile_pool(name=f"ep{layer}_lr", bufs=3) as lrp, \
                 tc.tile_pool(name=f"ep{layer}_St", bufs=3) as stp, \
                 tc.tile_pool(name=f"ep{layer}_out", bufs=2, space="PSUM") as outp_, \
                 tc.tile_pool(name=f"ep{layer}_den", bufs=2, space="PSUM") as denp, \
                 tc.tile_pool(name=f"ep{layer}_tr", bufs=1, space="PSUM") as trp, \
                 tc.tile_pool(name=f"ep{layer}_ade", bufs=2, space="PSUM") as adep, \
                 tc.tile_pool(name=f"ep{layer}_ps2", bufs=1, space="PSUM") as pp2:
                si_sb = idxp.tile([P, int(scol[L][-1])], i16, tag="si")
                nc.sync.dma_start(out=si_sb[:], in_=si_d_.ap()[:, :])
                ld_sb = idxp.tile([P, int(ldoff[L][-1])], i8, tag="ld")
                nc.sync.dma_start(out=ld_sb[:], in_=ld_d_.ap()[:, :])
                for b in range(nblk):
                    bbase = b * P
                    bm = min(P, ndst - bbase)
                    CL, CH = CLO[L][b], CHI[L][b]
                    CB = CL + CH
                    lo0 = int(ldoff[L][b])
                    rec = recp.tile([P, CB * ROW], bf16, tag="rec")
                    s0 = int(scol[L][b])
                    if CL:
                        nc.gpsimd.dma_gather(
                            rec[:, 0:CL * ROW].rearrange("p (c r) -> p c r",
                                                         r=ROW),
                            viewlo,
                            si_sb[:, s0:s0 + CL * 8],
                            CL * P, CL * P, ROW, single_packet=False,
                            queue_num=qctr[0] % 4)
                        qctr[0] += 1
                    if CH:
                        nc.gpsimd.dma_gather(
                            rec[:, CL * ROW:CB * ROW].rearrange(
                                "p (c r) -> p c r", r=ROW),
                            viewhi,
                            si_sb[:, s0 + CL * 8:s0 + CB * 8],
                            CH * P, CH * P, ROW, single_packet=False,
                            queue_num=qctr[0] % 4)
                        qctr[0] += 1
                    # dst alphas: contiguous local rows -> plain DMA + bcast
                    adv = advp.tile([P, 16], bf16, tag="adv")
                    nc.sync.dma_start(out=adv[:bm, :],
                                      in_=adt.ap()[bbase:bbase + bm, 0:16])
                    adb = advp.tile([P, H], bf16, tag="adb")
                    nc.vector.tensor_copy(adb[:], adv[:].bitcast(f32))
                    lrow = lrp.tile([P, CBMAX * P], bf16, tag="lrow")
                    nc.sync.dma_start(
                        out=lrow[:, 0:CB * P],
                        in_=lr_d_.ap()[b:b + 1, 0:CB * P]
                            .partition_broadcast(P))
                    St = stp.tile([P, CB * P], bf16, tag="St")
                    nc.vector.tensor_scalar(out=St[:, 0:CB * P],
                                            in0=lrow[:, 0:CB * P],
                                            scalar1=iotacf[:, 0:1],
                                            scalar2=None, op0=AL.is_equal)
                    ade = adep.tile([P, CB * H], f32, tag="ade")
                    for c in range(CB):
                        nc.tensor.matmul(ade[:, c * H:(c + 1) * H],
                                         lhsT=St[:, c * P:(c + 1) * P],
                                         rhs=adb[:], start=True, stop=True)
                    S = sp.tile([P, CB * P], bf16, tag="S")
                    nc.vector.tensor_tensor(
                        out=S[:].rearrange("p (c j) -> p c j", c=CB),
                        in0=iotaF[:, 0:CB * P].rearrange("p (c j) -> p c j",
                                                         c=CB),
                        in1=ld_sb[:, lo0:lo0 + CB].to_broadcast([P, CB, P]),
                        op=AL.is_equal)
                    recf = rec[:].bitcast(f32)
                    et = smp.tile([P, CB * H], f32, tag="et")
                    nc.vector.tensor_tensor(
                        out=et[:].rearrange("p (c h) -> p c h", c=CB),
                        in0=recf.rearrange("p (c r) -> p c r", c=CB)
                            [:, :, ASF:ASF + H],
                        in1=ade[:].rearrange("p (c h) -> p c h", c=CB),
                        op=AL.add)
                    lt = smp.tile([P, CB * H], f32, tag="lt")
                    nc.vector.scalar_tensor_tensor(out=lt[:], in0=et[:],
                                                   scalar=NEG, in1=et[:],
                                                   op0=AL.mult, op1=AL.max)
                    ex = smp.tile([P, CB * H], f32, tag="ex")
                    nc.scalar.activation(out=ex[:], in_=lt[:], func=AF.Exp)
                    exb = smp.tile([P, CB * H], bf16, tag="exb")
                    nc.vector.tensor_copy(exb[:], ex[:])
                    msgb = msgp.tile([P, CB * F], bf16, tag="msgb")
                    nc.vector.tensor_tensor(
                        out=msgb[:].rearrange("p (c h f) -> p c h f",
                                              c=CB, h=H),
                        in0=rec[:].rearrange("p (c r) -> p c r", c=CB)
                            [:, :, 0:F].rearrange("p c (h f) -> p c h f", h=H),
                        in1=ex[:].rearrange("p (c h) -> p c h", c=CB)
                            .to_broadcast([P, CB, H, Fh]),
                        op=AL.mult)
                    outp = outp_.tile([P, F], f32, tag="out")
                    den = denp.tile([P, H], f32, tag="den")
                    for c in range(CB):
                        nc.tensor.matmul(outp[:], lhsT=S[:, c * P:(c + 1) * P],
                                         rhs=msgb[:, c * F:(c + 1) * F],
                                         start=(c == 0), stop=(c == CB - 1))
                        nc.tensor.matmul(den[:], lhsT=S[:, c * P:(c + 1) * P],
                                         rhs=exb[:, c * H:(c + 1) * H],
                                         start=(c == 0), stop=(c == CB - 1))
                    r = epi.tile([P, H], f32, tag="r")
                    nc.vector.tensor_scalar(out=r[:], in0=den[:], scalar1=1e-16,
                                            scalar2=float(H), op0=AL.add,
                                            op1=AL.mult)
                    nc.vector.reciprocal(r[:], r[:])
                    tmp0 = epi.tile([P, F], f32, tag="tmp0")
                    nc.scalar.activation(out=tmp0[:], in_=outp[:], func=AF.Copy)
                    tmp = epi.tile([P, F], f32, tag="tmp")
                    nc.vector.tensor_tensor(
                        out=tmp[:].rearrange("p (h f) -> p h f", h=H),
                        in0=tmp0[:].rearrange("p (h f) -> p h f", h=H),
                        in1=r[:].to_broadcast([P, H, Fh]),
                        op=AL.mult)
                    acc = epi.tile([P, Fh], f32, tag="acc")
                    nc.vector.tensor_reduce(
                        out=acc[:], in_=tmp[:].rearrange("p (h f) -> p f h",
                                                         h=H),
                        axis=mybir.AxisListType.X, op=AL.add)
                    bs = b1s if layer == 1 else b2s
                    nc.vector.tensor_tensor(out=acc[:], in0=acc[:],
                                            in1=bs[:, 0:Fh], op=AL.add)
                    if layer == 1:
                        x2t = epi.tile([P, O1], f32, tag="x2")
                        nc.vector.tensor_scalar(out=x2t[:], in0=acc[:],
                                                scalar1=0.0, scalar2=None,
                                                op0=AL.max)
                        tr = trp.tile([O1, P], f32, tag="tr")
                        nc.tensor.transpose(out=tr[:], in_=x2t[:],
                                            identity=ident[:])
                        nc.vector.tensor_copy(xt2sb[:, bbase:bbase + P], tr[:])
                        ps2 = pp2.tile([P, 16 + F2], f32, tag="ps2")
                        nc.tensor.matmul(ps2[:bm, :],
                                         lhsT=xt2sb[:, bbase:bbase + bm],
                                         rhs=w2s[:, :], start=True, stop=True)
                        rec2 = epi.tile([P, F2 + 16], bf16, tag="rec2")
                        nc.scalar.activation(out=rec2[:bm, 0:F2],
                                             in_=ps2[:bm, 16:16 + F2],
                                             func=AF.Copy)
                        rec2f = rec2[:].bitcast(f32)
                        nc.vector.tensor_copy(rec2f[:bm, 128:128 + H],
                                              ps2[:bm, 0:H])
                        if b < nblk // 2:
                            nc.sync.dma_start(
                                out=hs2ownA.ap()[bbase:bbase + bm, 0:F2 + 16],
                                in_=rec2[:bm, :])
                        else:
                            nc.sync.dma_start(
                                out=hs2ownB.ap()[bbase - split:
                                                 bbase - split + bm,
                                                 0:F2 + 16],
                                in_=rec2[:bm, :])
                        adr2 = epi.tile([P, 16], bf16, tag="adr2")
                        adr2f = adr2[:].bitcast(f32)
                        nc.vector.tensor_copy(adr2f[:bm, 0:H],
                                              ps2[:bm, H:2 * H])
                        nc.sync.dma_start(out=ad2t.ap()[bbase:bbase + bm, 0:16],
                                          in_=adr2[:bm, :])
                        if b == nblk // 2 - 1:
                            if ncore > 1:
                                nc.gpsimd.collective_compute(
                                    "AllGather", mybir.AluOpType.bypass,
                                    replica_groups=[list(range(ncore))],
                                    ins=[hs2ownA.ap().opt()],
                                    outs=[hs2A.ap().opt()])
                            else:
                                nc.sync.dma_start(out=hs2A.ap()[:, :],
                                                  in_=hs2ownA.ap()[:, :])
                    else:
                        f = epi.tile([P, O2], f32, tag="f")
                        nc.vector.tensor_scalar(out=f[:], in0=acc[:],
                                                scalar1=0.0, scalar2=None,
                                                op0=AL.max)
                        nmx = epi.tile([P, 1], f32, tag="nmx")
                        nc.vector.tensor_reduce(out=nmx[:], in_=f[:],
                                                axis=mybir.AxisListType.X,
                                                op=AL.max, negate=True)
                        ef = epi.tile([P, O2], f32, tag="ef")
                        nc.scalar.activation(out=ef[:], in_=f[:], func=AF.Exp,
                                             bias=nmx[:, 0:1])
                        sm = epi.tile([P, 1], f32, tag="sm")
                        nc.vector.tensor_reduce(out=sm[:], in_=ef[:],
                                                axis=mybir.AxisListType.X,
                                                op=AL.add)
                        rs = epi.tile([P, 1], f32, tag="rs")
                        nc.vector.reciprocal(rs[:], sm[:])
                        nc.vector.tensor_scalar(out=ef[:], in0=ef[:],
                                                scalar1=rs[:, 0:1], scalar2=None,
                                                op0=AL.mult)
                        nc.sync.dma_start(out=outf_d.ap()[bbase:bbase + bm, :],
                                          in_=ef[:bm, :])

        edge_phase(1)

        if ncore > 1:
            nc.gpsimd.collective_compute(
                "AllGather", mybir.AluOpType.bypass,
                replica_groups=[list(range(ncore))],
                ins=[hs2ownB.ap().opt()], outs=[hs2B.ap().opt()])
        else:
            nc.sync.dma_start(out=hs2B.ap()[:, :], in_=hs2ownB.ap()[:, :])

        edge_phase(2)

    nc.compile()
    # Align each gather's SWDGE queue with its tile-assigned DMASW sem lane
    # (a sem lane is locked to one queue; lanes are assigned in scheduled
    # order, so the queue must be derived, not chosen up front).
    import re as _re
    for fn in nc.m.functions:
        for bb in fn.blocks:
            for inst in bb.instructions:
                if type(inst).__name__ == "InstDMAGatherAnt":
                    si = inst.sync_info
                    for u in (si.on_update if si is not None else []):
                        mm = _re.match(r"DMASW(\d+)_", u.ant_name or "")
                        if mm:
                            inst.queue_num = int(mm.group(1)) % 4
                            break
    return nc


def _prep_inputs(x, edge_index, W1, a_src1, a_dst1, b1, W2, a_src2, a_dst2, b2,
                 n, ncore, lohalf):
    ndst = n // ncore
    x = np.asarray(x, np.float32)
    W1 = np.asarray(W1, np.float32)
    W2 = np.asarray(W2, np.float32)
    As1 = np.einsum("hf,hfc->ch", np.asarray(a_src1, np.float32),
                    W1.reshape(H, O1, IN)).astype(np.float32)
    Ad1 = np.einsum("hf,hfc->ch", np.asarray(a_dst1, np.float32),
                    W1.reshape(H, O1, IN)).astype(np.float32)
    w1h = np.ascontiguousarray(W1.T)
    w1a = np.ascontiguousarray(np.concatenate([As1, Ad1], axis=1))
    As2 = np.einsum("hf,hfc->ch", np.asarray(a_src2, np.float32),
                    W2.reshape(H, O2, O1)).astype(np.float32)
    Ad2 = np.einsum("hf,hfc->ch", np.asarray(a_dst2, np.float32),
                    W2.reshape(H, O2, O1)).astype(np.float32)
    w2cat = np.ascontiguousarray(
        np.concatenate([As2, Ad2, W2.T], axis=1)).astype(np.float32)
    b1rep = np.ascontiguousarray(
        np.tile(np.asarray(b1, np.float32)[None, :], (P, 1)))
    b2rep = np.ascontiguousarray(
        np.tile(np.asarray(b2, np.float32)[None, :], (P, 1)))

    key, CLO, CHI, metas = _build_meta(np.asarray(edge_index), n, ncore,
                                       lohalf)
    xT = x.T
    in_maps = []
    for k in range(ncore):
        lo = k * ndst
        perm = np.concatenate([np.arange(lo, lo + ndst),
                               np.arange(0, lo), np.arange(lo + ndst, n)])
        sidx1, lrow1, ld1 = metas[k][0]
        sidx2, lrow2, ld2 = metas[k][1]
        in_maps.append({
            "xT": np.ascontiguousarray(xT[:, perm]).astype(ml_dtypes.bfloat16),
            "w1h": w1h.astype(ml_dtypes.bfloat16),
            "w1a": w1a.astype(ml_dtypes.bfloat16),
            "w2cat": w2cat,
            "b1rep": b1rep, "b2rep": b2rep,
            "sidx1": sidx1, "lr1": lrow1.astype(ml_dtypes.bfloat16),
            "ld1": ld1,
            "sidx2": sidx2, "lr2": lrow2.astype(ml_dtypes.bfloat16),
            "ld2": ld2,
        })
    return key, CLO, CHI, in_maps


def kernel(x, edge_index, W1, a_src1, a_dst1, b1, W2, a_src2, a_dst2, b2):
    key, CLO, CHI, in_maps = _prep_inputs(
        x, edge_index, W1, a_src1, a_dst1, b1, W2, a_src2, a_dst2, b2,
        N, NCORE, LOHALF)
    if key not in _cached:
        _cached[key] = _build_program(CLO, CHI, N, NCORE, LOHALF)
    nc = _cached[key]

    from concourse.bass_utils import run_bass_kernel_spmd
    kw = {}
    if os.environ.get("GAT_TRACE", "0") == "1":
        kw = dict(trace=True, tmpdir=os.environ.get("GAT_TRACE_DIR") or None)
    r = run_bass_kernel_spmd(nc, in_maps, list(range(NCORE)), **kw)
    global LAST_EXEC_NS, LAST_RESULT
    LAST_EXEC_NS = r.exec_time_ns
    LAST_RESULT = r
    out = np.concatenate([r.results[k]["outf"] for k in range(NCORE)], axis=0)
    return out.astype(np.float32)


LAST_EXEC_NS = None
LAST_RESULT = None


# revision 14
# speedup vs baseline: 2.6717x; 1.0714x over previous
"""GAT (2-layer, 8-head, mean over heads) Trainium2 Bass kernel, 8-core SPMD.

v3: dst-range sharding with dma_gather-based edge gathers (HW-verified
semantics; the v2 batched indirect-DMA turned out to stream contiguous rows
on HW). Design:

  * Per-node record tables in fp8 (h) with f32 alpha columns bit-cast into
    fp8 slots. Row strides are 256B-multiples (dma_gather constraint):
      hs1:   [h fp8 x512 | a_src f32 x8 @byte 512 | pad]    row 768 B
      hs2:   [h fp8 x256 | a_src f32 x8 @byte 256 | pad]    row 512 B
      adown: [a_dst f32 x8 | pad]                           row 256 B
  * dma_gather uses int16 indices (<32768), so the per-edge src gather is
    split into a lo-rows and hi-rows gather against two row-range views of
    the table; per-block chunk counts for each half are compile-time
    constants (max over cores).
  * Layer-1 tables are built in a PER-CORE PERMUTED row order (own dst range
    first, then the rest in natural order; the host permutes xT's columns),
    which makes "own rows" positional -> the dst-alpha table adown1 and its
    local (int16) dst indices are SPMD-clean.
  * Layer-2 records for the own dst range are computed from the transposed
    layer-1 activations and exchanged with one AllGather (natural row order);
    adown2 is written locally before the AllGather.
  * Per-block (not per-chunk) DVE/ACT processing; selection matrix S built
    with an int8 compare; scatter + denominators via per-chunk matmuls
    accumulating in PSUM.
"""

import os
import ml_dtypes
import numpy as np
from contextlib import ExitStack

N = 50000
E = 800000
H = 8
IN = 256
O1 = 64
O2 = 32
NCORE = 8
P = 128
NEG = 0.2
LOHALF = 32768

ROW1 = 768           # hs1 row, fp8 elements (768 B)
ROW2 = 384           # hs2 row (768 B)
ROWA = 128           # adown row (256 B)
F1 = H * O1          # 512
F2 = H * O2          # 256

_cached = {}


def _wrap_idx(flat, nidx):
    """dma_gather index layout: [128, nidx//16] int16, idx j at
    [j%16, j//16], replicated to all 8 groups of 16 partitions."""
    a = np.zeros((16, nidx // 16), np.int16)
    a[:, :] = np.asarray(flat, np.int16).reshape(nidx // 16, 16).T
    return np.tile(a, (8, 1))


def _build_meta(edge_index, n, ncore, lohalf):
    ndst = n // ncore
    nblk = (ndst + P - 1) // P
    split = (nblk // 2) * P          # local-row split for the two L2 tables
    src = np.concatenate([edge_index[0], np.arange(n, dtype=np.int64)])
    dst = np.concatenate([edge_index[1], np.arange(n, dtype=np.int64)])

    # per-core sorted edges + per-layer lo/hi split counts
    edges = []          # [core][block] -> (srow1, srow2, dloc) arrays
    nlo = np.zeros((2, ncore, nblk), np.int64)
    nhi = np.zeros((2, ncore, nblk), np.int64)
    for k in range(ncore):
        lo = k * ndst
        m = (dst >= lo) & (dst < lo + ndst)
        s_k = src[m]
        d_k = dst[m] - lo
        o = np.argsort(d_k, kind="stable")
        s_k = s_k[o]
        d_k = d_k[o]
        pos = np.zeros(n, np.int64)
        pos[lo:lo + ndst] = np.arange(ndst)
        others = np.concatenate([np.arange(0, lo), np.arange(lo + ndst, n)])
        pos[others] = ndst + np.arange(n - ndst)
        srow1 = pos[s_k]
        # L2 row ids in the A/B split tables: node g -> core k_g, local i;
        # A rows: k_g*split + i (i < split); B rows: k_g*(ndst-split) + i-split
        kg = s_k // ndst
        ig = s_k % ndst
        inA = ig < split
        srow2 = np.where(inA, kg * split + ig,
                         kg * (ndst - split) + (ig - split))
        blk = d_k // P
        per = []
        for b in range(nblk):
            mb = blk == b
            s1, s2, dl = srow1[mb], srow2[mb], d_k[mb]
            lo1 = s1 < lohalf
            lo2 = inA[mb]
            nlo[0, k, b] = lo1.sum()
            nhi[0, k, b] = (~lo1).sum()
            nlo[1, k, b] = lo2.sum()
            nhi[1, k, b] = (~lo2).sum()
            per.append((s1, s2, dl, lo2))
        edges.append(per)

    cdiv = lambda a, b: -(-a // b)
    CLO = [[int(cdiv(nlo[L, :, b].max(), P)) for b in range(nblk)]
           for L in (0, 1)]
    CHI = [[int(cdiv(nhi[L, :, b].max(), P)) for b in range(nblk)]
           for L in (0, 1)]
    CBb = [[CLO[L][b] + CHI[L][b] for b in range(nblk)] for L in (0, 1)]

    # idx tile column layouts (shared across cores)
    scol = [np.cumsum([0] + [(CLO[L][b] + CHI[L][b]) * 8 for b in range(nblk)])
            for L in (0, 1)]
    acol = [np.cumsum([0] + [CBb[L][b] * 8 for b in range(nblk)])
            for L in (0, 1)]

    CBMAX = max(max(CBb[0]), max(CBb[1]))
    metas = []
    for k in range(ncore):
        per_layer = {}
        for L in (0, 1):
            sidx = np.zeros((P, scol[L][-1]), np.int16)
            lrow = np.full((nblk, CBMAX * P), 200.0, np.float32)
            ld = np.full((P, sum(CBb[L])), -1, np.int8)
            ldoff = np.cumsum([0] + CBb[L])
            for b in range(nblk):
                s1, s2, dl, inA_b = edges[k][b]
                srow = s1 if L == 0 else s2
                isl = (srow < lohalf) if L == 0 else inA_b
                cl, ch = CLO[L][b], CHI[L][b]
                sl = np.zeros(cl * P, np.int64)
                sh = np.zeros(ch * P, np.int64)
                sl[:isl.sum()] = srow[isl]
                sh[:(~isl).sum()] = (srow[~isl] - lohalf) if L == 0 \
                    else srow[~isl]
                dfull = np.full((cl + ch) * P, -1, np.int64)
                dfull[:isl.sum()] = dl[isl]
                dfull[cl * P:cl * P + (~isl).sum()] = dl[~isl]
                c0 = scol[L][b]
                if cl:
                    sidx[:, c0:c0 + cl * 8] = _wrap_idx(sl, cl * P)
                if ch:
                    sidx[:, c0 + cl * 8:c0 + (cl + ch) * 8] = _wrap_idx(
                        sh, ch * P)
                lb = dfull - b * P
                lrow[b, 0:(cl + ch) * P] = np.where(lb >= 0, lb, 200)
                lb2 = lb.copy()
                lb2[dfull < 0] = -1
                ld[:, ldoff[b]:ldoff[b + 1]] = lb2.reshape(cl + ch, P).T
            per_layer[L] = (sidx, lrow, ld)
        metas.append(per_layer)
    key = (tuple(CLO[0]), tuple(CHI[0]), tuple(CLO[1]), tuple(CHI[1]))
    return key, CLO, CHI, metas


def _build_program(CLO, CHI, n, ncore, lohalf):
    import concourse.bacc as bacc
    import concourse.tile as tile
    from concourse import bass, mybir

    f32 = mybir.dt.float32
    bf16 = mybir.dt.bfloat16
    fp8 = mybir.dt.float8e4
    i16 = mybir.dt.int16
    i8 = mybir.dt.int8
    i32 = mybir.dt.int32
    AL = mybir.AluOpType
    AF = mybir.ActivationFunctionType

    ndst = n // ncore
    nblk = (ndst + P - 1) // P
    CBb = [[CLO[L][b] + CHI[L][b] for b in range(nblk)] for L in (0, 1)]
    scol = [np.cumsum([0] + [(CLO[L][b] + CHI[L][b]) * 8
                             for b in range(nblk)]) for L in (0, 1)]
    acol = [np.cumsum([0] + [CBb[L][b] * 8 for b in range(nblk)])
            for L in (0, 1)]
    ldoff = [np.cumsum([0] + CBb[L]) for L in (0, 1)]
    CBMAX = max(max(CBb[0]), max(CBb[1]))

    nc = bacc.Bacc("TRN2", target_bir_lowering=False, debug=False,
                   enable_asserts=True, num_devices=ncore,
                   num_swdge_queues=4)
    xT_d = nc.dram_tensor("xT", [IN, n], bf16, kind="ExternalInput")
    w1h_d = nc.dram_tensor("w1h", [IN, F1], bf16, kind="ExternalInput")
    w1a_d = nc.dram_tensor("w1a", [IN, 16], bf16, kind="ExternalInput")
    w2_d = nc.dram_tensor("w2cat", [O1, 16 + F2], f32, kind="ExternalInput")
    b1_d = nc.dram_tensor("b1rep", [P, O1], f32, kind="ExternalInput")
    b2_d = nc.dram_tensor("b2rep", [P, O2], f32, kind="ExternalInput")
    si1_d = nc.dram_tensor("sidx1", [P, int(scol[0][-1])], i16,
                           kind="ExternalInput")
    lr1_d = nc.dram_tensor("lr1", [nblk, CBMAX * P], bf16,
                           kind="ExternalInput")
    ld1_d = nc.dram_tensor("ld1", [P, int(ldoff[0][-1])], i8,
                           kind="ExternalInput")
    si2_d = nc.dram_tensor("sidx2", [P, int(scol[1][-1])], i16,
                           kind="ExternalInput")
    lr2_d = nc.dram_tensor("lr2", [nblk, CBMAX * P], bf16,
                           kind="ExternalInput")
    ld2_d = nc.dram_tensor("ld2", [P, int(ldoff[1][-1])], i8,
                           kind="ExternalInput")
    outf_d = nc.dram_tensor("outf", [ndst, O2], f32, kind="ExternalOutput")
    hs1 = nc.dram_tensor("hs1", [n, ROW1], fp8)
    ad1t = nc.dram_tensor("ad1t", [ndst, ROWA], bf16)
    ad2t = nc.dram_tensor("ad2t", [ndst, ROWA], bf16)
    split = (nblk // 2) * P
    shr = "Shared" if ncore > 4 else "Local"
    hs2ownA = nc.dram_tensor("hs2ownA", [split, ROW2], bf16)
    hs2ownB = nc.dram_tensor("hs2ownB", [ndst - split, ROW2], bf16)
    hs2A = nc.dram_tensor("hs2A", [ncore * split, ROW2], bf16, addr_space=shr)
    hs2B = nc.dram_tensor("hs2B", [ncore * (ndst - split), ROW2], bf16,
                          addr_space=shr)

    with tile.TileContext(nc) as tc, ExitStack() as ctx:
        cpool = ctx.enter_context(tc.tile_pool(name="const", bufs=1))

        iota_i = cpool.tile([P, P], i32, tag="io_i")
        nc.gpsimd.iota(iota_i[:], pattern=[[1, P]], base=0, channel_multiplier=0)
        iota8 = cpool.tile([P, P], i8, tag="io_8")
        nc.vector.tensor_copy(iota8[:], iota_i[:])
        iotaF = cpool.tile([P, CBMAX * P], i8, tag="iotaF")
        for c in range(CBMAX):
            nc.vector.tensor_copy(iotaF[:, c * P:(c + 1) * P], iota8[:])
        iotac_i = cpool.tile([P, 1], i32, tag="ioc_i")
        nc.gpsimd.iota(iotac_i[:], pattern=[[1, 1]], base=0, channel_multiplier=1)
        iotacf = cpool.tile([P, 1], f32, tag="ioc_f")
        nc.vector.tensor_copy(iotacf[:], iotac_i[:])
        iotaff = cpool.tile([P, P], f32, tag="io_f")
        nc.vector.tensor_copy(iotaff[:], iota_i[:])
        ident = cpool.tile([P, P], f32, tag="ident")
        nc.vector.tensor_scalar(out=ident[:], in0=iotaff[:], scalar1=iotacf[:, 0:1],
                                scalar2=None, op0=AL.is_equal)
        b1s = cpool.tile([P, O1], f32, tag="b1")
        nc.sync.dma_start(out=b1s[:], in_=b1_d.ap()[:, :])
        b2s = cpool.tile([P, O2], f32, tag="b2")
        nc.sync.dma_start(out=b2s[:], in_=b2_d.ap()[:, :])
        xt2sb = cpool.tile([O1, nblk * P], f32, tag="xt2")
        ones_row = cpool.tile([1, P], bf16, tag="ones")
        nc.vector.memset(ones_row[:], 1.0)
        w2s = cpool.tile([O1, 16 + F2], f32, tag="w2")
        nc.sync.dma_start(out=w2s[:], in_=w2_d.ap()[:, :])

        # ---------------- phase A1: layer-1 records (permuted order) --------
        with tc.tile_pool(name="pa_x", bufs=2) as xp, \
             tc.tile_pool(name="pa_w", bufs=1) as wp, \
             tc.tile_pool(name="pa_rec", bufs=3) as rp, \
             tc.tile_pool(name="pa_adr", bufs=3) as arp, \
             tc.tile_pool(name="pa_pm", bufs=3, space="PSUM") as pmp, \
             tc.tile_pool(name="pa_pa", bufs=3, space="PSUM") as pap:
            w1ha = wp.tile([P, F1], bf16, tag="w1ha")
            nc.sync.dma_start(out=w1ha[:], in_=w1h_d.ap()[0:P, :])
            w1hb = wp.tile([P, F1], bf16, tag="w1hb")
            nc.sync.dma_start(out=w1hb[:], in_=w1h_d.ap()[P:IN, :])
            w1aa = wp.tile([P, 16], bf16, tag="w1aa")
            nc.sync.dma_start(out=w1aa[:], in_=w1a_d.ap()[0:P, :])
            w1ab = wp.tile([P, 16], bf16, tag="w1ab")
            nc.sync.dma_start(out=w1ab[:], in_=w1a_d.ap()[P:IN, :])
            CHK = 2048
            for g0 in range(0, n, CHK):
                gw = min(CHK, n - g0)
                xa = xp.tile([P, CHK], bf16, tag="xa")
                nc.sync.dma_start(out=xa[:, :gw], in_=xT_d.ap()[0:P, g0:g0 + gw])
                xb = xp.tile([P, CHK], bf16, tag="xb")
                nc.sync.dma_start(out=xb[:, :gw], in_=xT_d.ap()[P:IN, g0:g0 + gw])
                for off in range(0, gw, P):
                    m = min(P, gw - off)
                    row0 = g0 + off
                    psh = pmp.tile([P, F1], f32, tag="psh")
                    nc.tensor.matmul(psh[:m, :], lhsT=xa[:, off:off + m],
                                     rhs=w1ha[:, :], start=True, stop=False)
                    nc.tensor.matmul(psh[:m, :], lhsT=xb[:, off:off + m],
                                     rhs=w1hb[:, :], start=False, stop=True)
                    psa = pap.tile([P, 16], f32, tag="psa")
                    nc.tensor.matmul(psa[:m, :], lhsT=xa[:, off:off + m],
                                     rhs=w1aa[:, :], start=True, stop=False)
                    nc.tensor.matmul(psa[:m, :], lhsT=xb[:, off:off + m],
                                     rhs=w1ab[:, :], start=False, stop=True)
                    rec = rp.tile([P, F1 + 32], fp8, tag="rec")
                    nc.scalar.activation(out=rec[:m, 0:F1], in_=psh[:m, :],
                                         func=AF.Copy)
                    recf = rec[:].bitcast(f32)
                    nc.vector.tensor_copy(recf[:m, 128:128 + H], psa[:m, 0:H])
                    nc.sync.dma_start(out=hs1.ap()[row0:row0 + m, 0:F1 + 32],
                                      in_=rec[:m, :])
                    if row0 < ndst:
                        mm = min(m, ndst - row0)
                        adr = arp.tile([P, 16], bf16, tag="adr")
                        adrf = adr[:].bitcast(f32)
                        nc.vector.tensor_copy(adrf[:mm, 0:H], psa[:mm, H:2 * H])
                        nc.sync.dma_start(out=ad1t.ap()[row0:row0 + mm, 0:16],
                                          in_=adr[:mm, :])

        # ---------------- edge phase ----------------------------------------
        qctr = [0]

        def edge_phase(layer):
            L = layer - 1
            if layer == 1:
                ROW, Fh, F, adt = ROW1, O1, F1, ad1t
                si_d_, lr_d_, ld_d_ = si1_d, lr1_d, ld1_d
                ASF = 128            # f32 index of a_src in row
                rdt = fp8
                viewlo = hs1.ap()[0:lohalf, :]
                viewhi = hs1.ap()[lohalf:n, :]
            else:
                ROW, Fh, F, adt = ROW2, O2, F2, ad2t
                si_d_, lr_d_, ld_d_ = si2_d, lr2_d, ld2_d
                ASF = 128
                rdt = bf16
                viewlo = hs2A.ap()[:, :]
                viewhi = hs2B.ap()[:, :]
            RF = ROW // 4            # f32 elements per row
            with tc.tile_pool(name=f"ep{layer}_idx", bufs=1) as idxp, \
                 tc.tile_pool(name=f"ep{layer}_rec", bufs=3) as recp, \
                 tc.tile_pool(name=f"ep{layer}_adv", bufs=3) as advp, \
                 tc.tile_pool(name=f"ep{layer}_S", bufs=3) as sp, \
                 tc.tile_pool(name=f"ep{layer}_sm", bufs=3) as smp, \
                 tc.tile_pool(name=f"ep{layer}_msg", bufs=2) as msgp, \
                 tc.tile_pool(name=f"ep{layer}_epi", bufs=2) as epi, \
                 tc.tile_pool(name=f"ep{layer}_lr", bufs=3) as lrp, \
                 tc.tile_pool(name=f"ep{layer}_St", bufs=3) as stp, \
                 tc.tile_pool(name=f"ep{layer}_out", bufs=2, space="PSUM") as outp_, \
                 tc.tile_pool(name=f"ep{layer}_den", bufs=2, space="PSUM") as denp, \
                 tc.tile_pool(name=f"ep{layer}_tr", bufs=1, space="PSUM") as trp, \
                 tc.tile_pool(name=f"ep{layer}_ade", bufs=2, space="PSUM") as adep, \
                 tc.tile_pool(name=f"ep{layer}_ps2", bufs=1, space="PSUM") as pp2:
                si_sb = idxp.tile([P, int(scol[L][-1])], i16, tag="si")
                nc.sync.dma_start(out=si_sb[:], in_=si_d_.ap()[:, :])
                ld_sb = idxp.tile([P, int(ldoff[L][-1])], i8, tag="ld")
                nc.sync.dma_start(out=ld_sb[:], in_=ld_d_.ap()[:, :])
                for b in range(nblk):
                    bbase = b * P
                    bm = min(P, ndst - bbase)
                    CL, CH = CLO[L][b], CHI[L][b]
                    CB = CL + CH
                    lo0 = int(ldoff[L][b])
                    rec = recp.tile([P, CB * ROW], rdt, tag="rec")
                    s0 = int(scol[L][b])
                    if CL:
                        nc.gpsimd.dma_gather(
                            rec[:, 0:CL * ROW].rearrange("p (c r) -> p c r",
                                                         r=ROW),
                            viewlo,
                            si_sb[:, s0:s0 + CL * 8],
                            CL * P, CL * P, ROW, single_packet=False,
                            queue_num=qctr[0] % 4)
                        qctr[0] += 1
                    if CH:
                        nc.gpsimd.dma_gather(
                            rec[:, CL * ROW:CB * ROW].rearrange(
                                "p (c r) -> p c r", r=ROW),
                            viewhi,
                            si_sb[:, s0 + CL * 8:s0 + CB * 8],
                            CH * P, CH * P, ROW, single_packet=False,
                            queue_num=qctr[0] % 4)
                        qctr[0] += 1
                    # dst alphas: contiguous local rows -> plain DMA + bcast
                    adv = advp.tile([P, 16], bf16, tag="adv")
                    nc.sync.dma_start(out=adv[:bm, :],
                                      in_=adt.ap()[bbase:bbase + bm, 0:16])
                    adb = advp.tile([P, H], bf16, tag="adb")
                    nc.vector.tensor_copy(adb[:], adv[:].bitcast(f32))
                    lrow = lrp.tile([P, CBMAX * P], bf16, tag="lrow")
                    nc.sync.dma_start(
                        out=lrow[:, 0:CB * P],
                        in_=lr_d_.ap()[b:b + 1, 0:CB * P]
                            .partition_broadcast(P))
                    St = stp.tile([P, CB * P], bf16, tag="St")
                    nc.vector.tensor_scalar(out=St[:, 0:CB * P],
                                            in0=lrow[:, 0:CB * P],
                                            scalar1=iotacf[:, 0:1],
                                            scalar2=None, op0=AL.is_equal)
                    ade = adep.tile([P, CB * H], f32, tag="ade")
                    for c in range(CB):
                        nc.tensor.matmul(ade[:, c * H:(c + 1) * H],
                                         lhsT=St[:, c * P:(c + 1) * P],
                                         rhs=adb[:], start=True, stop=True)
                    S = sp.tile([P, CB * P], bf16, tag="S")
                    nc.vector.tensor_tensor(
                        out=S[:].rearrange("p (c j) -> p c j", c=CB),
                        in0=iotaF[:, 0:CB * P].rearrange("p (c j) -> p c j",
                                                         c=CB),
                        in1=ld_sb[:, lo0:lo0 + CB].to_broadcast([P, CB, P]),
                        op=AL.is_equal)
                    recf = rec[:].bitcast(f32)
                    et = smp.tile([P, CB * H], f32, tag="et")
                    nc.vector.tensor_tensor(
                        out=et[:].rearrange("p (c h) -> p c h", c=CB),
                        in0=recf.rearrange("p (c r) -> p c r", c=CB)
                            [:, :, ASF:ASF + H],
                        in1=ade[:].rearrange("p (c h) -> p c h", c=CB),
                        op=AL.add)
                    lt = smp.tile([P, CB * H], f32, tag="lt")
                    nc.vector.scalar_tensor_tensor(out=lt[:], in0=et[:],
                                                   scalar=NEG, in1=et[:],
                                                   op0=AL.mult, op1=AL.max)
                    ex = smp.tile([P, CB * H], f32, tag="ex")
                    nc.scalar.activation(out=ex[:], in_=lt[:], func=AF.Exp)
                    exb = smp.tile([P, CB * H], bf16, tag="exb")
                    nc.vector.tensor_copy(exb[:], ex[:])
                    msgb = msgp.tile([P, CB * F], bf16, tag="msgb")
                    nc.vector.tensor_tensor(
                        out=msgb[:].rearrange("p (c h f) -> p c h f",
                                              c=CB, h=H),
                        in0=rec[:].rearrange("p (c r) -> p c r", c=CB)
                            [:, :, 0:F].rearrange("p c (h f) -> p c h f", h=H),
                        in1=ex[:].rearrange("p (c h) -> p c h", c=CB)
                            .to_broadcast([P, CB, H, Fh]),
                        op=AL.mult)
                    outp = outp_.tile([P, F], f32, tag="out")
                    den = denp.tile([P, H], f32, tag="den")
                    for c in range(CB):
                        nc.tensor.matmul(outp[:], lhsT=S[:, c * P:(c + 1) * P],
                                         rhs=msgb[:, c * F:(c + 1) * F],
                                         start=(c == 0), stop=(c == CB - 1))
                        nc.tensor.matmul(den[:], lhsT=S[:, c * P:(c + 1) * P],
                                         rhs=exb[:, c * H:(c + 1) * H],
                                         start=(c == 0), stop=(c == CB - 1))
                    r = epi.tile([P, H], f32, tag="r")
                    nc.vector.tensor_scalar(out=r[:], in0=den[:], scalar1=1e-16,
                                            scalar2=float(H), op0=AL.add,
                                            op1=AL.mult)
                    nc.vector.reciprocal(r[:], r[:])
                    tmp0 = epi.tile([P, F], f32, tag="tmp0")
                    nc.scalar.activation(out=tmp0[:], in_=outp[:], func=AF.Copy)
                    tmp = epi.tile([P, F], f32, tag="tmp")
                    nc.vector.tensor_tensor(
                        out=tmp[:].rearrange("p (h f) -> p h f", h=H),
                        in0=tmp0[:].rearrange("p (h f) -> p h f", h=H),
                        in1=r[:].to_broadcast([P, H, Fh]),
                        op=AL.mult)
                    acc = epi.tile([P, Fh], f32, tag="acc")
                    nc.vector.tensor_reduce(
                        out=acc[:], in_=tmp[:].rearrange("p (h f) -> p f h",
                                                         h=H),
                        axis=mybir.AxisListType.X, op=AL.add)
                    bs = b1s if layer == 1 else b2s
                    nc.vector.tensor_tensor(out=acc[:], in0=acc[:],
                                            in1=bs[:, 0:Fh], op=AL.add)
                    if layer == 1:
                        x2t = epi.tile([P, O1], f32, tag="x2")
                        nc.vector.tensor_scalar(out=x2t[:], in0=acc[:],
                                                scalar1=0.0, scalar2=None,
                                                op0=AL.max)
                        tr = trp.tile([O1, P], f32, tag="tr")
                        nc.tensor.transpose(out=tr[:], in_=x2t[:],
                                            identity=ident[:])
                        nc.vector.tensor_copy(xt2sb[:, bbase:bbase + P], tr[:])
                        ps2 = pp2.tile([P, 16 + F2], f32, tag="ps2")
                        nc.tensor.matmul(ps2[:bm, :],
                                         lhsT=xt2sb[:, bbase:bbase + bm],
                                         rhs=w2s[:, :], start=True, stop=True)
                        rec2 = epi.tile([P, F2 + 16], bf16, tag="rec2")
                        nc.scalar.activation(out=rec2[:bm, 0:F2],
                                             in_=ps2[:bm, 16:16 + F2],
                                             func=AF.Copy)
                        rec2f = rec2[:].bitcast(f32)
                        nc.vector.tensor_copy(rec2f[:bm, 128:128 + H],
                                              ps2[:bm, 0:H])
                        if b < nblk // 2:
                            nc.sync.dma_start(
                                out=hs2ownA.ap()[bbase:bbase + bm, 0:F2 + 16],
                                in_=rec2[:bm, :])
                        else:
                            nc.sync.dma_start(
                                out=hs2ownB.ap()[bbase - split:
                                                 bbase - split + bm,
                                                 0:F2 + 16],
                                in_=rec2[:bm, :])
                        adr2 = epi.tile([P, 16], bf16, tag="adr2")
                        adr2f = adr2[:].bitcast(f32)
                        nc.vector.tensor_copy(adr2f[:bm, 0:H],
                                              ps2[:bm, H:2 * H])
                        nc.sync.dma_start(out=ad2t.ap()[bbase:bbase + bm, 0:16],
                                          in_=adr2[:bm, :])
                        if b == nblk // 2 - 1:
                            if ncore > 1:
                                nc.gpsimd.collective_compute(
                                    "AllGather", mybir.AluOpType.bypass,
                                    replica_groups=[list(range(ncore))],
                                    ins=[hs2ownA.ap().opt()],
                                    outs=[hs2A.ap().opt()])
                            else:
                                nc.sync.dma_start(out=hs2A.ap()[:, :],
                                                  in_=hs2ownA.ap()[:, :])
                    else:
                        f = epi.tile([P, O2], f32, tag="f")
                        nc.vector.tensor_scalar(out=f[:], in0=acc[:],
                                                scalar1=0.0, scalar2=None,
                                                op0=AL.max)
                        nmx = epi.tile([P, 1], f32, tag="nmx")
                        nc.vector.tensor_reduce(out=nmx[:], in_=f[:],
                                                axis=mybir.AxisListType.X,
                                                op=AL.max, negate=True)
                        ef = epi.tile([P, O2], f32, tag="ef")
                        nc.scalar.activation(out=ef[:], in_=f[:], func=AF.Exp,
                                             bias=nmx[:, 0:1])
                        sm = epi.tile([P, 1], f32, tag="sm")
                        nc.vector.tensor_reduce(out=sm[:], in_=ef[:],
                                                axis=mybir.AxisListType.X,
                                                op=AL.add)
                        rs = epi.tile([P, 1], f32, tag="rs")
                        nc.vector.reciprocal(rs[:], sm[:])
                        nc.vector.tensor_scalar(out=ef[:], in0=ef[:],
                                                scalar1=rs[:, 0:1], scalar2=None,
                                                op0=AL.mult)
                        nc.sync.dma_start(out=outf_d.ap()[bbase:bbase + bm, :],
                                          in_=ef[:bm, :])

        edge_phase(1)

        if ncore > 1:
            nc.gpsimd.collective_compute(
                "AllGather", mybir.AluOpType.bypass,
                replica_groups=[list(range(ncore))],
                ins=[hs2ownB.ap().opt()], outs=[hs2B.ap().opt()])
        else:
            nc.sync.dma_start(out=hs2B.ap()[:, :], in_=hs2ownB.ap()[:, :])

        edge_phase(2)

    nc.compile()
    # Align each gather's SWDGE queue with its tile-assigned DMASW sem lane
    # (a sem lane is locked to one queue; lanes are assigned in scheduled
    # order, so the queue must be derived, not chosen up front).
    import re as _re
    for fn in nc.m.functions:
        for bb in fn.blocks:
            for inst in bb.instructions:
                if type(inst).__name__ == "InstDMAGatherAnt":
                    si = inst.sync_info
                    for u in (si.on_update if si is not None else []):
                        mm = _re.match(r"DMASW(\d+)_", u.ant_name or "")
                        if mm:
                            inst.queue_num = int(mm.group(1)) % 4
                            break
    return nc


def _prep_inputs(x, edge_index, W1, a_src1, a_dst1, b1, W2, a_src2, a_dst2, b2,
                 n, ncore, lohalf):
    ndst = n // ncore
    x = np.asarray(x, np.float32)
    W1 = np.asarray(W1, np.float32)
    W2 = np.asarray(W2, np.float32)
    As1 = np.einsum("hf,hfc->ch", np.asarray(a_src1, np.float32),
                    W1.reshape(H, O1, IN)).astype(np.float32)
    Ad1 = np.einsum("hf,hfc->ch", np.asarray(a_dst1, np.float32),
                    W1.reshape(H, O1, IN)).astype(np.float32)
    w1h = np.ascontiguousarray(W1.T)
    w1a = np.ascontiguousarray(np.concatenate([As1, Ad1], axis=1))
    As2 = np.einsum("hf,hfc->ch", np.asarray(a_src2, np.float32),
                    W2.reshape(H, O2, O1)).astype(np.float32)
    Ad2 = np.einsum("hf,hfc->ch", np.asarray(a_dst2, np.float32),
                    W2.reshape(H, O2, O1)).astype(np.float32)
    w2cat = np.ascontiguousarray(
        np.concatenate([As2, Ad2, W2.T], axis=1)).astype(np.float32)
    b1rep = np.ascontiguousarray(
        np.tile(np.asarray(b1, np.float32)[None, :], (P, 1)))
    b2rep = np.ascontiguousarray(
        np.tile(np.asarray(b2, np.float32)[None, :], (P, 1)))

    key, CLO, CHI, metas = _build_meta(np.asarray(edge_index), n, ncore,
                                       lohalf)
    xT = x.T
    in_maps = []
    for k in range(ncore):
        lo = k * ndst
        perm = np.concatenate([np.arange(lo, lo + ndst),
                               np.arange(0, lo), np.arange(lo + ndst, n)])
        sidx1, lrow1, ld1 = metas[k][0]
        sidx2, lrow2, ld2 = metas[k][1]
        in_maps.append({
            "xT": np.ascontiguousarray(xT[:, perm]).astype(ml_dtypes.bfloat16),
            "w1h": w1h.astype(ml_dtypes.bfloat16),
            "w1a": w1a.astype(ml_dtypes.bfloat16),
            "w2cat": w2cat,
            "b1rep": b1rep, "b2rep": b2rep,
            "sidx1": sidx1, "lr1": lrow1.astype(ml_dtypes.bfloat16),
            "ld1": ld1,
            "sidx2": sidx2, "lr2": lrow2.astype(ml_dtypes.bfloat16),
            "ld2": ld2,
        })
    return key, CLO, CHI, in_maps


def kernel(x, edge_index, W1, a_src1, a_dst1, b1, W2, a_src2, a_dst2, b2):
    key, CLO, CHI, in_maps = _prep_inputs(
        x, edge_index, W1, a_src1, a_dst1, b1, W2, a_src2, a_dst2, b2,
        N, NCORE, LOHALF)
    if key not in _cached:
        _cached[key] = _build_program(CLO, CHI, N, NCORE, LOHALF)
    nc = _cached[key]

    from concourse.bass_utils import run_bass_kernel_spmd
    kw = {}
    if os.environ.get("GAT_TRACE", "0") == "1":
        kw = dict(trace=True, tmpdir=os.environ.get("GAT_TRACE_DIR") or None)
    r = run_bass_kernel_spmd(nc, in_maps, list(range(NCORE)), **kw)
    global LAST_EXEC_NS, LAST_RESULT
    LAST_EXEC_NS = r.exec_time_ns
    LAST_RESULT = r
    out = np.concatenate([r.results[k]["outf"] for k in range(NCORE)], axis=0)
    return out.astype(np.float32)


LAST_EXEC_NS = None
LAST_RESULT = None
